# revision 48
# baseline (speedup 1.0000x reference)
"""Trainium2 Bass kernel for nn_EDM_task1 (scatter_memory).

Self-contained: takes FULL inputs, shards batch-parallel over 8 NeuronCores,
runs one SPMD Bass/Tile program per core, gathers FULL outputs.

Per batch row b the reference computes:
  obj_out = inp @ W_obj.T + b_obj                       [B, 128, 200]
  act_out = inp @ W_act.T + b_act                       [B, 128, 100]
  ne_out  = masked-mean-pool(inp) @ W_ne.T + b_ne       [B, 2000]
  scatter-max of selected act logits into classes; final = where(touched
  and positive, per-class max of positive logits, ne_out).

Mathematically (validated vs reference): with hostmask = objmask & (tgt!=-1),
  seg[b,c] = max(0, max_{n: hostmask, tgt=c} act_flat[b,n])
  final[b,c] = seg > 0 ? seg : ne_out      (selected logits are all > 0)

Device work split:
  - batches are data-parallel over cores (inp, act/obj GEMM, scatter-max)
  - W_ne is class-sharded over cores (each computes ne for ALL batches over
    its 250 classes); the trivial elementwise where() combine runs on host.

The scatter-max runs on-device as: per-partition GPSIMD local_scatter (by
destination partition) -> PE slab transposes -> per-partition local_scatter
(by class slot) -> DVE max-reduce over slots. Routing tables are pure
functions of integer inputs and are precomputed on host.
"""
from contextlib import ExitStack

import numpy as np

B, M, D = 64, 128, 2048
A_DIM, O_DIM, C_DIM = 100, 200, 2000
OA = O_DIM + A_DIM          # 300
NCORES = 8
BPC = B // NCORES           # 8 batches per core
KI = D // 128               # 16 contraction tiles
CSH = 256                   # padded ne class shard per core (250 real)

_prog_cache: dict = {}


def _build_routing(objmask: np.ndarray, AA: np.ndarray):
    """Vectorized host-side routing tables.

    ph1_idx [B,128,100] i16: per-(b,m) scatter dest = pstar*S + s, or -1.
    ph3_idx [B,128,128*S] i16: per-(b,pstar) dest = j*K + k at transposed
        position s*128 + m, or -1.  (class c = j*128 + pstar)
    """
    hm = ((objmask[:, :, None] > 0) & (AA.reshape(B, M, A_DIM) != -1))
    cls = AA.reshape(B, M, A_DIM).astype(np.int64)
    pstar = cls % 128
    jcls = cls // 128

    bb, mm, aa = np.meshgrid(np.arange(B), np.arange(M), np.arange(A_DIM),
                             indexing="ij")
    v = hm.ravel()
    bbv, mmv, aav = bb.ravel()[v], mm.ravel()[v], aa.ravel()[v]
    psv, jcv, clv = pstar.ravel()[v], jcls.ravel()[v], cls.ravel()[v]

    def group_rank(keys):
        # rank of each element within its key group, preserving input order
        order = np.argsort(keys, kind="stable")
        sk = keys[order]
        starts = np.r_[0, np.flatnonzero(sk[1:] != sk[:-1]) + 1]
        grp = np.zeros(len(sk), dtype=np.int64)
        grp[starts] = 1
        gid = np.cumsum(grp) - 1
        rank_sorted = np.arange(len(sk)) - starts[gid]
        ranks = np.empty(len(sk), dtype=np.int64)
        ranks[order] = rank_sorted
        return ranks

    s_slot = group_rank((bbv * M + mmv) * 128 + psv)
    k_slot = group_rank(bbv * C_DIM + clv)
    S = int(s_slot.max()) + 1
    K = int(k_slot.max()) + 1
    assert 128 * S <= 2046 and 16 * K <= 2046, (S, K)

    ph1_idx = np.full((B, M, A_DIM), -1, dtype=np.int16)
    ph1_idx[bbv, mmv, aav] = (psv * S + s_slot).astype(np.int16)
    ph3_idx = np.full((B, 128, 128 * S), -1, dtype=np.int16)
    ph3_idx[bbv, psv, s_slot * 128 + mmv] = (jcv * K + k_slot).astype(np.int16)
    return ph1_idx, ph3_idx, S, K


def _build_program(S: int, K: int):
    import concourse.mybir as mybir
    import concourse.tile as tile
    from concourse import bacc
    from concourse.masks import make_identity

    W1 = 128 * S
    W3 = 16 * K
    W1H = (S + 1) // 2 * 128      # first-half width of the transpose psum
    f32, f16, i16 = mybir.dt.float32, mybir.dt.float16, mybir.dt.int16

    nc = bacc.Bacc("TRN2", target_bir_lowering=False, debug=False)
    inpT_d = nc.dram_tensor("inpT", [128, 4, KI, 256], f32, kind="ExternalInput")
    w_aT_d = nc.dram_tensor("w_aT", [128, KI, A_DIM], f32, kind="ExternalInput")
    w_o16_d = nc.dram_tensor("w_o16", [128, KI, O_DIM], f16, kind="ExternalInput")
    b_oa_d = nc.dram_tensor("b_oa", [1, OA], f32, kind="ExternalInput")
    pooledT_d = nc.dram_tensor("pooledT", [128, KI, B], f16, kind="ExternalInput")
    w_neT_d = nc.dram_tensor("w_neT", [128, KI, CSH], f16, kind="ExternalInput")
    b_ne_d = nc.dram_tensor("b_ne", [1, CSH], f16, kind="ExternalInput")
    ph1_d = nc.dram_tensor("ph1", [128, BPC, A_DIM], i16, kind="ExternalInput")
    ph3_d = nc.dram_tensor("ph3", [128, BPC, W1], i16, kind="ExternalInput")
    obj_d = nc.dram_tensor("obj_out", [BPC, 128, O_DIM], f32, kind="ExternalOutput")
    act_d = nc.dram_tensor("act_out", [BPC, 128, A_DIM], f32, kind="ExternalOutput")
    seg_d = nc.dram_tensor("seg_out", [128, BPC, 16], f32, kind="ExternalOutput")
    ne_d = nc.dram_tensor("ne_out", [B, CSH], f32, kind="ExternalOutput")

    with tile.TileContext(nc) as tc, ExitStack() as ctx:
        const = ctx.enter_context(tc.tile_pool(name="const", bufs=1))
        work = ctx.enter_context(tc.tile_pool(name="work", bufs=5))
        ps_ac = ctx.enter_context(tc.tile_pool(name="ps_ac", bufs=3, space="PSUM"))
        ps_ob = ctx.enter_context(tc.tile_pool(name="ps_ob", bufs=1, space="PSUM"))
        ps_tp = ctx.enter_context(tc.tile_pool(name="ps_tp", bufs=2, space="PSUM"))
        ps_ne = ctx.enter_context(tc.tile_pool(name="ps_ne", bufs=1, space="PSUM"))

        ident16 = const.tile([128, 128], f16)
        make_identity(nc, ident16[:])
        ones16 = const.tile([1, B], f16)
        nc.vector.memset(ones16[:], 1.0)
        ones_row = const.tile([1, 128], f32)
        nc.vector.memset(ones_row[:], 1.0)

        # ---- resident inputs (few multi-level-AP DMAs, critical-first) ----
        # order matters: the HWDGE queue is FIFO, so load what batch 0's
        # GEMM + scatter need first (w_oa halves, inpT stripe 0, ph1).
        w_a_sb = const.tile([128, KI, A_DIM], f32)
        w_o16_sb = const.tile([128, KI, O_DIM], f16)
        b_oa_sb = const.tile([1, OA], f32)
        inpT_sb = const.tile([128, KI, BPC * 128], f32)
        x16_sb = const.tile([128, KI, BPC * 128], f16)
        ph1_sb = const.tile([128, BPC, A_DIM], i16)
        pooledT_sb = const.tile([128, KI, B], f16)
        w_ne_sb = const.tile([128, KI, CSH], f16)
        b_ne_sb = const.tile([1, CSH], f16)

        nc.sync.dma_start(w_a_sb[:], w_aT_d[:])
        nc.sync.dma_start(b_oa_sb[:], b_oa_d[:])
        nc.sync.dma_start(inpT_sb[:, :, 0:256], inpT_d[:, 0])
        nc.sync.dma_start(ph1_sb[:], ph1_d[:])
        ph3_sb = const.tile([128, BPC, W1], i16)
        nc.sync.dma_start(ph3_sb[:, :2, :], ph3_d[:, :2, :])
        nc.sync.dma_start(w_o16_sb[:], w_o16_d[:])
        nc.sync.dma_start(inpT_sb[:, :, 256:512], inpT_d[:, 1])
        nc.sync.dma_start(ph3_sb[:, 2:5, :], ph3_d[:, 2:5, :])
        nc.sync.dma_start(inpT_sb[:, :, 512:768], inpT_d[:, 2])
        nc.sync.dma_start(inpT_sb[:, :, 768:1024], inpT_d[:, 3])
        nc.sync.dma_start(ph3_sb[:, 5:, :], ph3_d[:, 5:, :])
        nc.sync.dma_start(pooledT_sb[:], pooledT_d[:])
        nc.sync.dma_start(w_ne_sb[:], w_neT_d[:])
        nc.sync.dma_start(b_ne_sb[:], b_ne_d[:])

        # x16 = fp16(inpT) built on device (ACT engine), one cast per stripe
        for g in range(4):
            cols = slice(g * 256, (g + 1) * 256)
            nc.scalar.copy(x16_sb[:, :, cols], inpT_sb[:, :, cols])

        # bias broadcast row -> [128, OA] tile (built once via PE)
        bias_ps = ps_ne.tile([128, OA], f32, tag="bias")
        nc.tensor.matmul(bias_ps[:], ones_row[:1, :], b_oa_sb[:],
                         start=True, stop=True)
        bias_bc = const.tile([128, OA], f32)
        nc.scalar.copy(bias_bc[:], bias_ps[:])

        seg_all = const.tile([128, BPC, 16], f32)

        # ---- per-batch pipeline, software-staged so each in-order engine
        # never stalls on a peer: stage A = GEMM + evac + phase-1 scatter,
        # stage B = slab transposes (PE) + evac, stage C = phase-3 scatter +
        # reduce. B lags A by 2 batches, C lags B by 1.
        t1s, t2s = {}, {}

        def stage_a(b):
            tcols = slice(b * 128, (b + 1) * 128)
            act_ps = ps_ac.tile([128, A_DIM], f32, tag="ac")
            for ki in range(KI):
                nc.tensor.matmul(act_ps[:], inpT_sb[:, ki, tcols],
                                 w_a_sb[:, ki, :],
                                 start=(ki == 0), stop=(ki == KI - 1))
            act_sb = work.tile([128, A_DIM], f32)
            nc.vector.tensor_add(act_sb[:], act_ps[:], bias_bc[:, O_DIM:])
            nc.sync.dma_start(act_d[b], act_sb[:])
            act16 = work.tile([128, A_DIM], f16)
            nc.scalar.copy(act16[:], act_sb[:])
            t1 = work.tile([128, W1], f16, tag="t1")
            nc.gpsimd.local_scatter(t1[:], act16[:], ph1_sb[:, b, :],
                                    channels=128, num_elems=W1, num_idxs=A_DIM)
            t1s[b] = t1

        def stage_b(b):
            # slab transposes: t2[p, s*128+m] = t1[m, p*S+s]
            t1v = t1s.pop(b)[:].rearrange("p (q s) -> p s q", s=S)
            t2 = work.tile([128, W1], f16, tag="t2")
            for lo, hi in ((0, (S + 1) // 2), ((S + 1) // 2, S)):
                if lo >= hi:
                    continue
                tp = ps_tp.tile([128, W1H], f16, tag="tp")
                for s in range(lo, hi):
                    o = (s - lo) * 128
                    nc.tensor.transpose(tp[:, o:o + 128], t1v[:, s, :], ident16[:])
                nc.vector.tensor_copy(t2[:, lo * 128:hi * 128],
                                      tp[:, :(hi - lo) * 128])
            t2s[b] = t2

        def stage_c(b):
            t3 = work.tile([128, W3], f16, tag="t3")
            nc.gpsimd.local_scatter(t3[:], t2s.pop(b)[:], ph3_sb[:, b, :],
                                    channels=128, num_elems=W3, num_idxs=W1)
            nc.vector.tensor_reduce(out=seg_all[:, b, :],
                                    in_=t3[:].rearrange("p (j k) -> p j k", k=K),
                                    axis=mybir.AxisListType.X,
                                    op=mybir.AluOpType.max)

        def stage_d(b):
            tcols = slice(b * 128, (b + 1) * 128)
            obj_ps = ps_ob.tile([128, O_DIM], f32, tag="ob")
            for ki in range(KI):
                nc.tensor.matmul(obj_ps[:], x16_sb[:, ki, tcols],
                                 w_o16_sb[:, ki, :],
                                 start=(ki == 0), stop=(ki == KI - 1))
            obj_sb = work.tile([128, O_DIM], f32, tag="obj_sb")
            nc.vector.tensor_add(obj_sb[:], obj_ps[:], bias_bc[:, :O_DIM])
            nc.sync.dma_start(obj_d[b], obj_sb[:])

        for b in range(BPC + 2):
            if b < BPC:
                stage_a(b)
            if 1 <= b < BPC + 1:
                stage_b(b - 1)
            if b >= 2:
                stage_c(b - 2)
            if 2 <= b < BPC + 2:
                stage_d(b - 2)
            if b == BPC:
                # non-exist head, class-sharded: ne[all 64 b, 256 shard].
                # Emitted late so its PE chain fills the pipeline drain (its
                # weights also arrive last in the DMA queue).
                ne_ps = ps_ne.tile([B, CSH], f32)
                nc.tensor.matmul(ne_ps[:], ones16[:], b_ne_sb[:],
                                 start=True, stop=False)
                for ki in range(KI):
                    nc.tensor.matmul(ne_ps[:], pooledT_sb[:, ki, :],
                                     w_ne_sb[:, ki, :],
                                     start=False, stop=(ki == KI - 1))
                ne_sb = work.tile([B, CSH], f32)
                nc.scalar.copy(ne_sb[:], ne_ps[:])
                nc.sync.dma_start(ne_d[:], ne_sb[:])
        nc.sync.dma_start(seg_d[:], seg_all[:])

    nc.compile()
    return nc


def _prep_host(inputs):
    inp = np.ascontiguousarray(inputs["inp"], dtype=np.float32)
    objmask = np.asarray(inputs["objmask"], dtype=np.float32)
    AA = np.asarray(inputs["AAidxs_tgts"]).astype(np.int64)
    W_obj = np.asarray(inputs["W_obj"], dtype=np.float32)
    b_obj = np.asarray(inputs["b_obj"], dtype=np.float32)
    W_act = np.asarray(inputs["W_act"], dtype=np.float32)
    b_act = np.asarray(inputs["b_act"], dtype=np.float32)
    W_ne = np.asarray(inputs["W_ne"], dtype=np.float32)
    b_ne = np.asarray(inputs["b_ne"], dtype=np.float32)

    ph1_idx, ph3_idx, S, K = _build_routing(objmask, AA)

    # weights, chunked [KI, 128, cols]
    w_aT = np.ascontiguousarray(
        W_act.T.reshape(KI, 128, A_DIM).transpose(1, 0, 2))
    w_o16 = np.ascontiguousarray(
        W_obj.T.astype(np.float16).reshape(KI, 128, O_DIM).transpose(1, 0, 2))
    b_oa = np.concatenate([b_obj, b_act])[None, :]

    # normalized-mask pooling on host (pure input function)
    nmask = objmask / objmask.sum(axis=1, keepdims=True)
    pooled = np.einsum("bmd,bm->bd", inp, nmask).astype(np.float32)
    pooledT = np.ascontiguousarray(
        pooled.T.reshape(KI, 128, B).transpose(1, 0, 2)).astype(np.float16)

    in_maps = []
    for c in range(NCORES):
        bs = slice(c * BPC, (c + 1) * BPC)
        cs = slice(c * (C_DIM // NCORES), (c + 1) * (C_DIM // NCORES))
        inpT = inp[bs].reshape(BPC * 128, D).T.reshape(KI, 128, 4, 256)
        inpT = np.ascontiguousarray(inpT.transpose(1, 2, 0, 3))
        w_neT = np.zeros((128, KI, CSH), np.float16)
        w_neT[:, :, :C_DIM // NCORES] = W_ne[cs].T.astype(np.float16).reshape(
            KI, 128, C_DIM // NCORES).transpose(1, 0, 2)
        b_ne_p = np.zeros((1, CSH), np.float16)
        b_ne_p[0, :C_DIM // NCORES] = b_ne[cs].astype(np.float16)
        in_maps.append({
            "inpT": inpT,
            "w_aT": w_aT,
            "w_o16": w_o16,
            "b_oa": b_oa,
            "pooledT": pooledT,
            "w_neT": w_neT,
            "b_ne": b_ne_p,
            "ph1": np.ascontiguousarray(ph1_idx[bs].transpose(1, 0, 2)),
            "ph3": np.ascontiguousarray(ph3_idx[bs].transpose(1, 0, 2)),
        })
    return in_maps, S, K


def _assemble(outs):
    """Combine per-core results into full outputs (+ host where-combine)."""
    obj_out = np.concatenate([np.asarray(o["obj_out"]) for o in outs])
    act_out = np.concatenate([np.asarray(o["act_out"]) for o in outs])
    seg = np.concatenate(
        [np.transpose(np.asarray(o["seg_out"]), (1, 2, 0)).reshape(BPC, 2048)
         for o in outs])[:, :C_DIM]
    ne = np.concatenate(
        [np.asarray(o["ne_out"])[:, :C_DIM // NCORES] for o in outs], axis=1)
    final = np.where(seg > 0, seg, ne).astype(np.float32)
    return final, act_out, obj_out


def kernel(**inputs) -> tuple:
    from concourse.bass_utils import run_bass_kernel_spmd

    in_maps, S, K = _prep_host(inputs)
    key = (S, K)
    if key not in _prog_cache:
        _prog_cache[key] = _build_program(S, K)
    nc = _prog_cache[key]

    res = run_bass_kernel_spmd(nc, in_maps, core_ids=list(range(NCORES)))
    return _assemble(res.results)


# revision 51
# speedup vs baseline: 1.0416x; 1.0416x over previous
"""Trainium2 Bass kernel for nn_EDM_task1 (scatter_memory).

Self-contained: takes FULL inputs, shards batch-parallel over 8 NeuronCores,
runs one SPMD Bass/Tile program per core, gathers FULL outputs.

Per batch row b the reference computes:
  obj_out = inp @ W_obj.T + b_obj                       [B, 128, 200]
  act_out = inp @ W_act.T + b_act                       [B, 128, 100]
  ne_out  = masked-mean-pool(inp) @ W_ne.T + b_ne       [B, 2000]
  scatter-max of selected act logits into classes; final = where(touched
  and positive, per-class max of positive logits, ne_out).

Mathematically (validated vs reference): with hostmask = objmask & (tgt!=-1),
  seg[b,c] = max(0, max_{n: hostmask, tgt=c} act_flat[b,n])
  final[b,c] = seg > 0 ? seg : ne_out      (selected logits are all > 0)

Device work split:
  - batches are data-parallel over cores (inp, act/obj GEMM, scatter-max)
  - W_ne is class-sharded over cores (each computes ne for ALL batches over
    its 250 classes); the trivial elementwise where() combine runs on host.

The scatter-max runs on-device as: per-partition GPSIMD local_scatter (by
destination partition) -> PE slab transposes -> per-partition local_scatter
(by class slot) -> DVE max-reduce over slots. Routing tables are pure
functions of integer inputs and are precomputed on host.
"""
from contextlib import ExitStack

import numpy as np

B, M, D = 64, 128, 2048
A_DIM, O_DIM, C_DIM = 100, 200, 2000
OA = O_DIM + A_DIM          # 300
NCORES = 8
BPC = B // NCORES           # 8 batches per core
KI = D // 128               # 16 contraction tiles
CSH = 256                   # padded ne class shard per core (250 real)

_prog_cache: dict = {}


def _build_routing(objmask: np.ndarray, AA: np.ndarray):
    """Vectorized host-side routing tables.

    ph1_idx [B,128,100] i16: per-(b,m) scatter dest = pstar*S + s, or -1.
    ph3_idx [B,128,128*S] i16: per-(b,pstar) dest = j*K + k at transposed
        position s*128 + m, or -1.  (class c = j*128 + pstar)
    """
    hm = ((objmask[:, :, None] > 0) & (AA.reshape(B, M, A_DIM) != -1))
    cls = AA.reshape(B, M, A_DIM).astype(np.int64)

    bb, mm, aa = np.meshgrid(np.arange(B), np.arange(M), np.arange(A_DIM),
                             indexing="ij")
    v = hm.ravel()
    bbv, mmv, aav = bb.ravel()[v], mm.ravel()[v], aa.ravel()[v]
    clv = cls.ravel()[v]

    # the class -> (partition, j-slot) map is free (host un-permutes seg);
    # pick the candidate position permutation minimizing the phase-1 slab
    # count S = max same-destination-partition multiplicity per (b, m) row.
    row_key = bbv * M + mmv
    best = None
    rng = np.random.default_rng(12345)
    for trial in range(32):
        pos = (np.arange(C_DIM) if trial == 0
               else rng.permutation(C_DIM)).astype(np.int64)
        ps_t = pos[clv] % 128
        key = row_key * 128 + ps_t
        _, counts = np.unique(key, return_counts=True)
        s_max = int(counts.max())
        if best is None or s_max < best[0]:
            best = (s_max, pos)
        if best[0] <= 6:
            break
    pos = best[1]
    psv = pos[clv] % 128
    jcv = pos[clv] // 128

    def group_rank(keys):
        # rank of each element within its key group, preserving input order
        order = np.argsort(keys, kind="stable")
        sk = keys[order]
        starts = np.r_[0, np.flatnonzero(sk[1:] != sk[:-1]) + 1]
        grp = np.zeros(len(sk), dtype=np.int64)
        grp[starts] = 1
        gid = np.cumsum(grp) - 1
        rank_sorted = np.arange(len(sk)) - starts[gid]
        ranks = np.empty(len(sk), dtype=np.int64)
        ranks[order] = rank_sorted
        return ranks

    s_slot = group_rank((bbv * M + mmv) * 128 + psv)
    k_slot = group_rank(bbv * C_DIM + clv)
    S = int(s_slot.max()) + 1
    K = int(k_slot.max()) + 1
    assert 128 * S <= 2046 and 16 * K <= 2046, (S, K)

    ph1_idx = np.full((B, M, A_DIM), -1, dtype=np.int16)
    ph1_idx[bbv, mmv, aav] = (psv * S + s_slot).astype(np.int16)
    ph3_idx = np.full((B, 128, 128 * S), -1, dtype=np.int16)
    ph3_idx[bbv, psv, s_slot * 128 + mmv] = (jcv * K + k_slot).astype(np.int16)
    return ph1_idx, ph3_idx, S, K, pos


def _build_program(S: int, K: int):
    import concourse.mybir as mybir
    import concourse.tile as tile
    from concourse import bacc
    from concourse.masks import make_identity

    W1 = 128 * S
    W3 = 16 * K
    W1H = (S + 1) // 2 * 128      # first-half width of the transpose psum
    f32, f16, i16 = mybir.dt.float32, mybir.dt.float16, mybir.dt.int16

    nc = bacc.Bacc("TRN2", target_bir_lowering=False, debug=False)
    inpT_d = nc.dram_tensor("inpT", [128, 4, KI, 256], f32, kind="ExternalInput")
    w_aT_d = nc.dram_tensor("w_aT", [128, KI, A_DIM], f32, kind="ExternalInput")
    w_o16_d = nc.dram_tensor("w_o16", [128, KI, O_DIM], f16, kind="ExternalInput")
    b_oa_d = nc.dram_tensor("b_oa", [1, OA], f32, kind="ExternalInput")
    pooledT_d = nc.dram_tensor("pooledT", [128, KI, B], f16, kind="ExternalInput")
    w_neT_d = nc.dram_tensor("w_neT", [128, KI, CSH], f16, kind="ExternalInput")
    b_ne_d = nc.dram_tensor("b_ne", [1, CSH], f16, kind="ExternalInput")
    ph1_d = nc.dram_tensor("ph1", [128, BPC, A_DIM], i16, kind="ExternalInput")
    ph3_d = nc.dram_tensor("ph3", [128, BPC, W1], i16, kind="ExternalInput")
    obj_d = nc.dram_tensor("obj_out", [BPC, 128, O_DIM], f32, kind="ExternalOutput")
    act_d = nc.dram_tensor("act_out", [BPC, 128, A_DIM], f32, kind="ExternalOutput")
    seg_d = nc.dram_tensor("seg_out", [128, BPC, 16], f32, kind="ExternalOutput")
    ne_d = nc.dram_tensor("ne_out", [B, CSH], f32, kind="ExternalOutput")

    with tile.TileContext(nc) as tc, ExitStack() as ctx:
        const = ctx.enter_context(tc.tile_pool(name="const", bufs=1))
        work = ctx.enter_context(tc.tile_pool(name="work", bufs=5))
        ps_ac = ctx.enter_context(tc.tile_pool(name="ps_ac", bufs=3, space="PSUM"))
        ps_ob = ctx.enter_context(tc.tile_pool(name="ps_ob", bufs=1, space="PSUM"))
        ps_tp = ctx.enter_context(tc.tile_pool(name="ps_tp", bufs=2, space="PSUM"))
        ps_ne = ctx.enter_context(tc.tile_pool(name="ps_ne", bufs=1, space="PSUM"))

        ident16 = const.tile([128, 128], f16)
        make_identity(nc, ident16[:])
        ones16 = const.tile([1, B], f16)
        nc.vector.memset(ones16[:], 1.0)
        ones_row = const.tile([1, 128], f32)
        nc.vector.memset(ones_row[:], 1.0)

        # ---- resident inputs (few multi-level-AP DMAs, critical-first) ----
        # order matters: the HWDGE queue is FIFO, so load what batch 0's
        # GEMM + scatter need first (w_oa halves, inpT stripe 0, ph1).
        w_a_sb = const.tile([128, KI, A_DIM], f32)
        w_o16_sb = const.tile([128, KI, O_DIM], f16)
        b_oa_sb = const.tile([1, OA], f32)
        inpT_sb = const.tile([128, KI, BPC * 128], f32)
        x16_sb = const.tile([128, KI, BPC * 128], f16)
        ph1_sb = const.tile([128, BPC, A_DIM], i16)
        pooledT_sb = const.tile([128, KI, B], f16)
        w_ne_sb = const.tile([128, KI, CSH], f16)
        b_ne_sb = const.tile([1, CSH], f16)

        nc.sync.dma_start(w_a_sb[:], w_aT_d[:])
        nc.sync.dma_start(b_oa_sb[:], b_oa_d[:])
        nc.sync.dma_start(inpT_sb[:, :, 0:256], inpT_d[:, 0])
        nc.sync.dma_start(ph1_sb[:], ph1_d[:])
        ph3_sb = const.tile([128, BPC, W1], i16)
        nc.sync.dma_start(ph3_sb[:, :2, :], ph3_d[:, :2, :])
        nc.sync.dma_start(w_o16_sb[:], w_o16_d[:])
        nc.sync.dma_start(inpT_sb[:, :, 256:512], inpT_d[:, 1])
        nc.sync.dma_start(ph3_sb[:, 2:5, :], ph3_d[:, 2:5, :])
        nc.sync.dma_start(inpT_sb[:, :, 512:768], inpT_d[:, 2])
        nc.sync.dma_start(inpT_sb[:, :, 768:1024], inpT_d[:, 3])
        nc.sync.dma_start(ph3_sb[:, 5:, :], ph3_d[:, 5:, :])
        nc.sync.dma_start(pooledT_sb[:], pooledT_d[:])
        nc.sync.dma_start(w_ne_sb[:], w_neT_d[:])
        nc.sync.dma_start(b_ne_sb[:], b_ne_d[:])

        # bias broadcast row -> [128, OA] tile (built once via PE)
        bias_ps = ps_ne.tile([128, OA], f32, tag="bias")
        nc.tensor.matmul(bias_ps[:], ones_row[:1, :], b_oa_sb[:],
                         start=True, stop=True)
        bias_bc = const.tile([128, OA], f32)
        nc.scalar.copy(bias_bc[:], bias_ps[:])

        seg_all = const.tile([128, BPC, 16], f32)

        # ---- per-batch pipeline, software-staged so each in-order engine
        # never stalls on a peer: stage A = GEMM + evac + phase-1 scatter,
        # stage B = slab transposes (PE) + evac, stage C = phase-3 scatter +
        # reduce. B lags A by 2 batches, C lags B by 1.
        t1s, t2s = {}, {}

        def stage_a(b):
            tcols = slice(b * 128, (b + 1) * 128)
            act_ps = ps_ac.tile([128, A_DIM], f32, tag="ac")
            for ki in range(KI):
                nc.tensor.matmul(act_ps[:], inpT_sb[:, ki, tcols],
                                 w_a_sb[:, ki, :],
                                 start=(ki == 0), stop=(ki == KI - 1))
            act_sb = work.tile([128, A_DIM], f32)
            nc.vector.tensor_add(act_sb[:], act_ps[:], bias_bc[:, O_DIM:])
            nc.sync.dma_start(act_d[b], act_sb[:])
            act16 = work.tile([128, A_DIM], f16)
            nc.scalar.copy(act16[:], act_sb[:])
            t1 = work.tile([128, W1], f16, tag="t1")
            nc.gpsimd.local_scatter(t1[:], act16[:], ph1_sb[:, b, :],
                                    channels=128, num_elems=W1, num_idxs=A_DIM)
            t1s[b] = t1

        def stage_b(b):
            # slab transposes: t2[p, s*128+m] = t1[m, p*S+s]
            t1v = t1s.pop(b)[:].rearrange("p (q s) -> p s q", s=S)
            t2 = work.tile([128, W1], f16, tag="t2")
            for lo, hi in ((0, (S + 1) // 2), ((S + 1) // 2, S)):
                if lo >= hi:
                    continue
                tp = ps_tp.tile([128, W1H], f16, tag="tp")
                for s in range(lo, hi):
                    o = (s - lo) * 128
                    nc.tensor.transpose(tp[:, o:o + 128], t1v[:, s, :], ident16[:])
                nc.vector.tensor_copy(t2[:, lo * 128:hi * 128],
                                      tp[:, :(hi - lo) * 128])
            t2s[b] = t2

        def stage_c(b):
            t3 = work.tile([128, W3], f16, tag="t3")
            nc.gpsimd.local_scatter(t3[:], t2s.pop(b)[:], ph3_sb[:, b, :],
                                    channels=128, num_elems=W3, num_idxs=W1)
            nc.vector.tensor_reduce(out=seg_all[:, b, :],
                                    in_=t3[:].rearrange("p (j k) -> p j k", k=K),
                                    axis=mybir.AxisListType.X,
                                    op=mybir.AluOpType.max)

        def stage_d(b):
            tcols = slice(b * 128, (b + 1) * 128)
            obj_ps = ps_ob.tile([128, O_DIM], f32, tag="ob")
            for ki in range(KI):
                nc.tensor.matmul(obj_ps[:], x16_sb[:, ki, tcols],
                                 w_o16_sb[:, ki, :],
                                 start=(ki == 0), stop=(ki == KI - 1))
            obj_sb = work.tile([128, O_DIM], f32, tag="obj_sb")
            nc.vector.tensor_add(obj_sb[:], obj_ps[:], bias_bc[:, :O_DIM])
            nc.sync.dma_start(obj_d[b], obj_sb[:])

        for b in range(BPC + 2):
            if b < BPC:
                stage_a(b)
            if b % 2 == 1 and b // 2 < 4:
                g = b // 2
                cols = slice(g * 256, (g + 1) * 256)
                nc.scalar.copy(x16_sb[:, :, cols], inpT_sb[:, :, cols])
            if 1 <= b < BPC + 1:
                stage_b(b - 1)
            if b >= 2:
                stage_c(b - 2)
            if 2 <= b < BPC + 2:
                stage_d(b - 2)
            if b == BPC:
                # non-exist head, class-sharded: ne[all 64 b, 256 shard].
                # Emitted late so its PE chain fills the pipeline drain (its
                # weights also arrive last in the DMA queue).
                ne_ps = ps_ne.tile([B, CSH], f32)
                nc.tensor.matmul(ne_ps[:], ones16[:], b_ne_sb[:],
                                 start=True, stop=False)
                for ki in range(KI):
                    nc.tensor.matmul(ne_ps[:], pooledT_sb[:, ki, :],
                                     w_ne_sb[:, ki, :],
                                     start=False, stop=(ki == KI - 1))
                ne_sb = work.tile([B, CSH], f32)
                nc.scalar.copy(ne_sb[:], ne_ps[:])
                nc.sync.dma_start(ne_d[:], ne_sb[:])
        nc.sync.dma_start(seg_d[:], seg_all[:])

    nc.compile()
    return nc


def _prep_host(inputs):
    inp = np.ascontiguousarray(inputs["inp"], dtype=np.float32)
    objmask = np.asarray(inputs["objmask"], dtype=np.float32)
    AA = np.asarray(inputs["AAidxs_tgts"]).astype(np.int64)
    W_obj = np.asarray(inputs["W_obj"], dtype=np.float32)
    b_obj = np.asarray(inputs["b_obj"], dtype=np.float32)
    W_act = np.asarray(inputs["W_act"], dtype=np.float32)
    b_act = np.asarray(inputs["b_act"], dtype=np.float32)
    W_ne = np.asarray(inputs["W_ne"], dtype=np.float32)
    b_ne = np.asarray(inputs["b_ne"], dtype=np.float32)

    ph1_idx, ph3_idx, S, K, pos = _build_routing(objmask, AA)

    # weights, chunked [KI, 128, cols]
    w_aT = np.ascontiguousarray(
        W_act.T.reshape(KI, 128, A_DIM).transpose(1, 0, 2))
    w_o16 = np.ascontiguousarray(
        W_obj.T.astype(np.float16).reshape(KI, 128, O_DIM).transpose(1, 0, 2))
    b_oa = np.concatenate([b_obj, b_act])[None, :]

    # normalized-mask pooling on host (pure input function)
    nmask = objmask / objmask.sum(axis=1, keepdims=True)
    pooled = np.einsum("bmd,bm->bd", inp, nmask).astype(np.float32)
    pooledT = np.ascontiguousarray(
        pooled.T.reshape(KI, 128, B).transpose(1, 0, 2)).astype(np.float16)

    in_maps = []
    for c in range(NCORES):
        bs = slice(c * BPC, (c + 1) * BPC)
        cs = slice(c * (C_DIM // NCORES), (c + 1) * (C_DIM // NCORES))
        inpT = inp[bs].reshape(BPC * 128, D).T.reshape(KI, 128, 4, 256)
        inpT = np.ascontiguousarray(inpT.transpose(1, 2, 0, 3))
        w_neT = np.zeros((128, KI, CSH), np.float16)
        w_neT[:, :, :C_DIM // NCORES] = W_ne[cs].T.astype(np.float16).reshape(
            KI, 128, C_DIM // NCORES).transpose(1, 0, 2)
        b_ne_p = np.zeros((1, CSH), np.float16)
        b_ne_p[0, :C_DIM // NCORES] = b_ne[cs].astype(np.float16)
        in_maps.append({
            "inpT": inpT,
            "w_aT": w_aT,
            "w_o16": w_o16,
            "b_oa": b_oa,
            "pooledT": pooledT,
            "w_neT": w_neT,
            "b_ne": b_ne_p,
            "ph1": np.ascontiguousarray(ph1_idx[bs].transpose(1, 0, 2)),
            "ph3": np.ascontiguousarray(ph3_idx[bs].transpose(1, 0, 2)),
        })
    return in_maps, S, K, pos


def _assemble(outs, pos):
    """Combine per-core results into full outputs (+ host where-combine)."""
    obj_out = np.concatenate([np.asarray(o["obj_out"]) for o in outs])
    act_out = np.concatenate([np.asarray(o["act_out"]) for o in outs])
    # seg_out[p, b, j] holds the class at permuted position j*128 + p
    seg_pos = np.concatenate(
        [np.transpose(np.asarray(o["seg_out"]), (1, 2, 0)).reshape(BPC, 2048)
         for o in outs])
    seg = seg_pos[:, pos]
    ne = np.concatenate(
        [np.asarray(o["ne_out"])[:, :C_DIM // NCORES] for o in outs], axis=1)
    final = np.where(seg > 0, seg, ne).astype(np.float32)
    return final, act_out, obj_out


def kernel(**inputs) -> tuple:
    from concourse.bass_utils import run_bass_kernel_spmd

    in_maps, S, K, pos = _prep_host(inputs)
    key = (S, K)
    if key not in _prog_cache:
        _prog_cache[key] = _build_program(S, K)
    nc = _prog_cache[key]

    res = run_bass_kernel_spmd(nc, in_maps, core_ids=list(range(NCORES)))
    return _assemble(res.results, pos)


# revision 54
# speedup vs baseline: 1.0574x; 1.0151x over previous
"""Trainium2 Bass kernel for nn_EDM_task1 (scatter_memory).

Self-contained: takes FULL inputs, shards batch-parallel over 8 NeuronCores,
runs one SPMD Bass/Tile program per core, gathers FULL outputs.

Per batch row b the reference computes:
  obj_out = inp @ W_obj.T + b_obj                       [B, 128, 200]
  act_out = inp @ W_act.T + b_act                       [B, 128, 100]
  ne_out  = masked-mean-pool(inp) @ W_ne.T + b_ne       [B, 2000]
  scatter-max of selected act logits into classes; final = where(touched
  and positive, per-class max of positive logits, ne_out).

Mathematically (validated vs reference): with hostmask = objmask & (tgt!=-1),
  seg[b,c] = max(0, max_{n: hostmask, tgt=c} act_flat[b,n])
  final[b,c] = seg > 0 ? seg : ne_out      (selected logits are all > 0)

Device work split:
  - batches are data-parallel over cores (inp, act/obj GEMM, scatter-max)
  - W_ne is class-sharded over cores (each computes ne for ALL batches over
    its 250 classes); the trivial elementwise where() combine runs on host.

The scatter-max runs on-device as: per-partition GPSIMD local_scatter (by
destination partition) -> PE slab transposes -> per-partition local_scatter
(by class slot) -> DVE max-reduce over slots. Routing tables are pure
functions of integer inputs and are precomputed on host.
"""
from contextlib import ExitStack

import numpy as np

B, M, D = 64, 128, 2048
A_DIM, O_DIM, C_DIM = 100, 200, 2000
OA = O_DIM + A_DIM          # 300
NCORES = 8
BPC = B // NCORES           # 8 batches per core
KI = D // 128               # 16 contraction tiles
CSH = 256                   # padded ne class shard per core (250 real)

_prog_cache: dict = {}


def _build_routing(objmask: np.ndarray, AA: np.ndarray):
    """Vectorized host-side routing tables.

    ph1_idx [B,128,100] i16: per-(b,m) scatter dest = pstar*S + s, or -1.
    ph3_idx [B,128,128*S] i16: per-(b,pstar) dest = j*K + k at transposed
        position s*128 + m, or -1.  (class c = j*128 + pstar)
    """
    hm = ((objmask[:, :, None] > 0) & (AA.reshape(B, M, A_DIM) != -1))
    cls = AA.reshape(B, M, A_DIM).astype(np.int64)

    bb, mm, aa = np.meshgrid(np.arange(B), np.arange(M), np.arange(A_DIM),
                             indexing="ij")
    v = hm.ravel()
    bbv, mmv, aav = bb.ravel()[v], mm.ravel()[v], aa.ravel()[v]
    clv = cls.ravel()[v]

    # the class -> (partition, j-slot) map is free (host un-permutes seg);
    # pick the candidate position permutation minimizing the phase-1 slab
    # count S = max same-destination-partition multiplicity per (b, m) row.
    row_key = bbv * M + mmv
    best = None
    rng = np.random.default_rng(12345)
    for trial in range(32):
        pos = (np.arange(C_DIM) if trial == 0
               else rng.permutation(C_DIM)).astype(np.int64)
        ps_t = pos[clv] % 128
        key = row_key * 128 + ps_t
        _, counts = np.unique(key, return_counts=True)
        s_max = int(counts.max())
        if best is None or s_max < best[0]:
            best = (s_max, pos)
        if best[0] <= 6:
            break
    pos = best[1]
    psv = pos[clv] % 128
    jcv = pos[clv] // 128

    def group_rank(keys):
        # rank of each element within its key group, preserving input order
        order = np.argsort(keys, kind="stable")
        sk = keys[order]
        starts = np.r_[0, np.flatnonzero(sk[1:] != sk[:-1]) + 1]
        grp = np.zeros(len(sk), dtype=np.int64)
        grp[starts] = 1
        gid = np.cumsum(grp) - 1
        rank_sorted = np.arange(len(sk)) - starts[gid]
        ranks = np.empty(len(sk), dtype=np.int64)
        ranks[order] = rank_sorted
        return ranks

    s_slot = group_rank((bbv * M + mmv) * 128 + psv)
    k_slot = group_rank(bbv * C_DIM + clv)
    S = int(s_slot.max()) + 1
    K = int(k_slot.max()) + 1
    assert 128 * S <= 2046 and 16 * K <= 2046, (S, K)

    ph1_idx = np.full((B, M, A_DIM), -1, dtype=np.int16)
    ph1_idx[bbv, mmv, aav] = (psv * S + s_slot).astype(np.int16)
    ph3_idx = np.full((B, 128, 128 * S), -1, dtype=np.int16)
    ph3_idx[bbv, psv, s_slot * 128 + mmv] = (jcv * K + k_slot).astype(np.int16)
    return ph1_idx, ph3_idx, S, K, pos


def _build_program(S: int, K: int):
    import concourse.mybir as mybir
    import concourse.tile as tile
    from concourse import bacc
    from concourse.masks import make_identity

    W1 = 128 * S
    W3 = 16 * K
    W1H = (S + 1) // 2 * 128      # first-half width of the transpose psum
    f32, f16, i16 = mybir.dt.float32, mybir.dt.float16, mybir.dt.int16

    nc = bacc.Bacc("TRN2", target_bir_lowering=False, debug=False)
    inpT_d = nc.dram_tensor("inpT", [128, 4, KI, 256], f32, kind="ExternalInput")
    w_aT_d = nc.dram_tensor("w_aT", [128, KI, A_DIM], f32, kind="ExternalInput")
    w_o16_d = nc.dram_tensor("w_o16", [128, KI, O_DIM], f16, kind="ExternalInput")
    b_oa_d = nc.dram_tensor("b_oa", [1, OA], f32, kind="ExternalInput")
    pooledT_d = nc.dram_tensor("pooledT", [128, KI, B], f16, kind="ExternalInput")
    w_neT_d = nc.dram_tensor("w_neT", [128, KI, CSH], f16, kind="ExternalInput")
    b_ne_d = nc.dram_tensor("b_ne", [1, CSH], f16, kind="ExternalInput")
    ph1_d = nc.dram_tensor("ph1", [128, BPC, A_DIM], i16, kind="ExternalInput")
    ph3_d = nc.dram_tensor("ph3", [128, BPC, W1], i16, kind="ExternalInput")
    obj_d = nc.dram_tensor("obj_out", [BPC, 128, O_DIM], f32, kind="ExternalOutput")
    act_d = nc.dram_tensor("act_out", [BPC, 128, A_DIM], f32, kind="ExternalOutput")
    seg_d = nc.dram_tensor("seg_out", [128, BPC, 16], f32, kind="ExternalOutput")
    ne_d = nc.dram_tensor("ne_out", [B, CSH], f32, kind="ExternalOutput")

    with tile.TileContext(nc) as tc, ExitStack() as ctx:
        const = ctx.enter_context(tc.tile_pool(name="const", bufs=1))
        work = ctx.enter_context(tc.tile_pool(name="work", bufs=5))
        ps_ac = ctx.enter_context(tc.tile_pool(name="ps_ac", bufs=3, space="PSUM"))
        ps_ob = ctx.enter_context(tc.tile_pool(name="ps_ob", bufs=1, space="PSUM"))
        ps_tp = ctx.enter_context(tc.tile_pool(name="ps_tp", bufs=2, space="PSUM"))
        ps_ne = ctx.enter_context(tc.tile_pool(name="ps_ne", bufs=1, space="PSUM"))

        ident16 = const.tile([128, 128], f16)
        make_identity(nc, ident16[:])
        ones16 = const.tile([1, B], f16)
        nc.vector.memset(ones16[:], 1.0)
        ones_row = const.tile([1, 128], f32)
        nc.vector.memset(ones_row[:], 1.0)

        # ---- resident inputs (few multi-level-AP DMAs, critical-first) ----
        # order matters: the HWDGE queue is FIFO, so load what batch 0's
        # GEMM + scatter need first (w_oa halves, inpT stripe 0, ph1).
        w_a_sb = const.tile([128, KI, A_DIM], f32)
        w_o16_sb = const.tile([128, KI, O_DIM], f16)
        b_oa_sb = const.tile([1, OA], f32)
        inpT_sb = const.tile([128, KI, BPC * 128], f32)
        x16_sb = const.tile([128, KI, BPC * 128], f16)
        ph1_sb = const.tile([128, BPC, A_DIM], i16)
        pooledT_sb = const.tile([128, KI, B], f16)
        w_ne_sb = const.tile([128, KI, CSH], f16)
        b_ne_sb = const.tile([1, CSH], f16)

        nc.sync.dma_start(w_a_sb[:], w_aT_d[:])
        nc.sync.dma_start(b_oa_sb[:], b_oa_d[:])
        nc.sync.dma_start(inpT_sb[:, :, 0:256], inpT_d[:, 0])
        nc.sync.dma_start(ph1_sb[:], ph1_d[:])
        ph3_sb = const.tile([128, BPC, W1], i16)
        nc.sync.dma_start(ph3_sb[:, :2, :], ph3_d[:, :2, :])
        nc.sync.dma_start(w_o16_sb[:], w_o16_d[:])
        nc.sync.dma_start(inpT_sb[:, :, 256:512], inpT_d[:, 1])
        nc.sync.dma_start(ph3_sb[:, 2:5, :], ph3_d[:, 2:5, :])
        nc.sync.dma_start(inpT_sb[:, :, 512:768], inpT_d[:, 2])
        nc.sync.dma_start(inpT_sb[:, :, 768:1024], inpT_d[:, 3])
        nc.sync.dma_start(ph3_sb[:, 5:, :], ph3_d[:, 5:, :])
        nc.sync.dma_start(pooledT_sb[:], pooledT_d[:])
        nc.sync.dma_start(w_ne_sb[:], w_neT_d[:])
        nc.sync.dma_start(b_ne_sb[:], b_ne_d[:])

        # bias broadcast row -> [128, OA] tile (built once via PE)
        bias_ps = ps_ne.tile([128, OA], f32, tag="bias")
        nc.tensor.matmul(bias_ps[:], ones_row[:1, :], b_oa_sb[:],
                         start=True, stop=True)
        bias_bc = const.tile([128, OA], f32)
        nc.scalar.copy(bias_bc[:], bias_ps[:])

        seg_all = const.tile([128, BPC, 16], f32)

        # ---- per-batch pipeline, software-staged so each in-order engine
        # never stalls on a peer: stage A = GEMM + evac + phase-1 scatter,
        # stage B = slab transposes (PE) + evac, stage C = phase-3 scatter +
        # reduce. B lags A by 2 batches, C lags B by 1.
        t1s, t2s = {}, {}

        def stage_a(b):
            tcols = slice(b * 128, (b + 1) * 128)
            act_ps = ps_ac.tile([128, A_DIM], f32, tag="ac")
            for ki in range(KI):
                nc.tensor.matmul(act_ps[:], inpT_sb[:, ki, tcols],
                                 w_a_sb[:, ki, :],
                                 start=(ki == 0), stop=(ki == KI - 1))
            act_sb = work.tile([128, A_DIM], f32)
            nc.vector.tensor_add(act_sb[:], act_ps[:], bias_bc[:, O_DIM:])
            nc.sync.dma_start(act_d[b], act_sb[:])
            act16 = work.tile([128, A_DIM], f16)
            nc.scalar.copy(act16[:], act_sb[:])
            t1 = work.tile([128, W1], f16, tag="t1")
            nc.gpsimd.local_scatter(t1[:], act16[:], ph1_sb[:, b, :],
                                    channels=128, num_elems=W1, num_idxs=A_DIM)
            t1s[b] = t1

        def stage_b(b):
            # slab transposes: t2[p, s*128+m] = t1[m, p*S+s]
            t1v = t1s.pop(b)[:].rearrange("p (q s) -> p s q", s=S)
            t2 = work.tile([128, W1], f16, tag="t2")
            for lo, hi in ((0, (S + 1) // 2), ((S + 1) // 2, S)):
                if lo >= hi:
                    continue
                tp = ps_tp.tile([128, W1H], f16, tag="tp")
                for s in range(lo, hi):
                    o = (s - lo) * 128
                    nc.tensor.transpose(tp[:, o:o + 128], t1v[:, s, :], ident16[:])
                nc.vector.tensor_copy(t2[:, lo * 128:hi * 128],
                                      tp[:, :(hi - lo) * 128])
            t2s[b] = t2

        def stage_c(b):
            t3 = work.tile([128, W3], f16, tag="t3")
            nc.gpsimd.local_scatter(t3[:], t2s.pop(b)[:], ph3_sb[:, b, :],
                                    channels=128, num_elems=W3, num_idxs=W1)
            nc.vector.tensor_reduce(out=seg_all[:, b, :],
                                    in_=t3[:].rearrange("p (j k) -> p j k", k=K),
                                    axis=mybir.AxisListType.X,
                                    op=mybir.AluOpType.max)

        def stage_d(b):
            tcols = slice(b * 128, (b + 1) * 128)
            obj_ps = ps_ob.tile([128, O_DIM], f32, tag="ob")
            for ki in range(KI):
                nc.tensor.matmul(obj_ps[:], x16_sb[:, ki, tcols],
                                 w_o16_sb[:, ki, :],
                                 start=(ki == 0), stop=(ki == KI - 1))
            obj_sb = work.tile([128, O_DIM], f32, tag="obj_sb")
            nc.vector.tensor_add(obj_sb[:], obj_ps[:], bias_bc[:, :O_DIM])
            nc.sync.dma_start(obj_d[b], obj_sb[:])

        for b in range(BPC + 2):
            if b < BPC:
                stage_a(b)
            if b % 2 == 1 and b // 2 < 4:
                g = b // 2
                cols = slice(g * 256, (g + 1) * 256)
                nc.scalar.copy(x16_sb[:, :, cols], inpT_sb[:, :, cols])
            if 1 <= b < BPC + 1:
                stage_b(b - 1)
            if b >= 2:
                stage_c(b - 2)
            if 2 <= b < BPC + 2:
                stage_d(b - 2)
            if b == BPC:
                # non-exist head, class-sharded: ne[all 64 b, 256 shard].
                # Emitted late so its PE chain fills the pipeline drain (its
                # weights also arrive last in the DMA queue).
                ne_ps = ps_ne.tile([B, CSH], f32)
                nc.tensor.matmul(ne_ps[:], ones16[:], b_ne_sb[:],
                                 start=True, stop=False)
                for ki in range(KI):
                    nc.tensor.matmul(ne_ps[:], pooledT_sb[:, ki, :],
                                     w_ne_sb[:, ki, :],
                                     start=False, stop=(ki == KI - 1))
                ne_sb = work.tile([B, CSH], f32)
                nc.scalar.copy(ne_sb[:], ne_ps[:])
                nc.sync.dma_start(ne_d[:], ne_sb[:])
        nc.sync.dma_start(seg_d[:], seg_all[:])

    nc.compile()
    return nc


def _prep_host(inputs):
    inp = np.ascontiguousarray(inputs["inp"], dtype=np.float32)
    objmask = np.asarray(inputs["objmask"], dtype=np.float32)
    AA = np.asarray(inputs["AAidxs_tgts"]).astype(np.int64)
    W_obj = np.asarray(inputs["W_obj"], dtype=np.float32)
    b_obj = np.asarray(inputs["b_obj"], dtype=np.float32)
    W_act = np.asarray(inputs["W_act"], dtype=np.float32)
    b_act = np.asarray(inputs["b_act"], dtype=np.float32)
    W_ne = np.asarray(inputs["W_ne"], dtype=np.float32)
    b_ne = np.asarray(inputs["b_ne"], dtype=np.float32)

    ph1_idx, ph3_idx, S, K, pos = _build_routing(objmask, AA)

    # weights, chunked [KI, 128, cols]
    w_aT = np.ascontiguousarray(
        W_act.T.reshape(KI, 128, A_DIM).transpose(1, 0, 2))
    w_o16 = np.ascontiguousarray(
        W_obj.T.astype(np.float16).reshape(KI, 128, O_DIM).transpose(1, 0, 2))
    b_oa = np.concatenate([b_obj, b_act])[None, :]

    # normalized-mask pooling on host (pure input function)
    nmask = objmask / objmask.sum(axis=1, keepdims=True)
    pooled = np.einsum("bmd,bm->bd", inp, nmask).astype(np.float32)
    pooledT = np.ascontiguousarray(
        pooled.T.reshape(KI, 128, B).transpose(1, 0, 2)).astype(np.float16)

    in_maps = []
    for c in range(NCORES):
        bs = slice(c * BPC, (c + 1) * BPC)
        cs = slice(c * (C_DIM // NCORES), (c + 1) * (C_DIM // NCORES))
        inpT = inp[bs].reshape(BPC * 128, D).T.reshape(KI, 128, 4, 256)
        inpT = np.ascontiguousarray(inpT.transpose(1, 2, 0, 3))
        w_neT = np.zeros((128, KI, CSH), np.float16)
        w_neT[:, :, :C_DIM // NCORES] = W_ne[cs].T.astype(np.float16).reshape(
            KI, 128, C_DIM // NCORES).transpose(1, 0, 2)
        b_ne_p = np.zeros((1, CSH), np.float16)
        b_ne_p[0, :C_DIM // NCORES] = b_ne[cs].astype(np.float16)
        in_maps.append({
            "inpT": inpT,
            "w_aT": w_aT,
            "w_o16": w_o16,
            "b_oa": b_oa,
            "pooledT": pooledT,
            "w_neT": w_neT,
            "b_ne": b_ne_p,
            "ph1": np.ascontiguousarray(ph1_idx[bs].transpose(1, 0, 2)),
            "ph3": np.ascontiguousarray(ph3_idx[bs].transpose(1, 0, 2)),
        })
    return in_maps, S, K, pos


def _assemble(outs, pos):
    """Combine per-core results into full outputs (+ host where-combine)."""
    obj_out = np.concatenate([np.asarray(o["obj_out"]) for o in outs])
    act_out = np.concatenate([np.asarray(o["act_out"]) for o in outs])
    # seg_out[p, b, j] holds the class at permuted position j*128 + p
    seg_pos = np.concatenate(
        [np.transpose(np.asarray(o["seg_out"]), (1, 2, 0)).reshape(BPC, 2048)
         for o in outs])
    seg = seg_pos[:, pos]
    ne = np.concatenate(
        [np.asarray(o["ne_out"])[:, :C_DIM // NCORES] for o in outs], axis=1)
    final = np.where(seg > 0, seg, ne).astype(np.float32)
    return final, act_out, obj_out


def kernel(**inputs) -> tuple:
    from concourse.bass_utils import run_bass_kernel_spmd

    in_maps, S, K, pos = _prep_host(inputs)
    key = (S, K)
    if key not in _prog_cache:
        _prog_cache[key] = _build_program(S, K)
    nc = _prog_cache[key]

    res = run_bass_kernel_spmd(nc, in_maps, core_ids=list(range(NCORES)))
    return _assemble(res.results, pos)


# revision 55
# speedup vs baseline: 1.0798x; 1.0213x over previous
"""Trainium2 Bass kernel for nn_EDM_task1 (scatter_memory).

Self-contained: takes FULL inputs, shards batch-parallel over 8 NeuronCores,
runs one SPMD Bass/Tile program per core, gathers FULL outputs.

Per batch row b the reference computes:
  obj_out = inp @ W_obj.T + b_obj                       [B, 128, 200]
  act_out = inp @ W_act.T + b_act                       [B, 128, 100]
  ne_out  = masked-mean-pool(inp) @ W_ne.T + b_ne       [B, 2000]
  scatter-max of selected act logits into classes; final = where(touched
  and positive, per-class max of positive logits, ne_out).

Mathematically (validated vs reference): with hostmask = objmask & (tgt!=-1),
  seg[b,c] = max(0, max_{n: hostmask, tgt=c} act_flat[b,n])
  final[b,c] = seg > 0 ? seg : ne_out      (selected logits are all > 0)

Device work split:
  - batches are data-parallel over cores (inp, act/obj GEMM, scatter-max)
  - W_ne is class-sharded over cores (each computes ne for ALL batches over
    its 250 classes); the trivial elementwise where() combine runs on host.

The scatter-max runs on-device as: per-partition GPSIMD local_scatter (by
destination partition) -> PE slab transposes -> per-partition local_scatter
(by class slot) -> DVE max-reduce over slots. Routing tables are pure
functions of integer inputs and are precomputed on host.
"""
from contextlib import ExitStack

import numpy as np

B, M, D = 64, 128, 2048
A_DIM, O_DIM, C_DIM = 100, 200, 2000
OA = O_DIM + A_DIM          # 300
NCORES = 8
BPC = B // NCORES           # 8 batches per core
KI = D // 128               # 16 contraction tiles
CSH = 256                   # padded ne class shard per core (250 real)

_prog_cache: dict = {}


def _build_routing(objmask: np.ndarray, AA: np.ndarray):
    """Vectorized host-side routing tables.

    ph1_idx [B,128,100] i16: per-(b,m) scatter dest = pstar*S + s, or -1.
    ph3_idx [B,128,128*S] i16: per-(b,pstar) dest = j*K + k at transposed
        position s*128 + m, or -1.  (class c = j*128 + pstar)
    """
    hm = ((objmask[:, :, None] > 0) & (AA.reshape(B, M, A_DIM) != -1))
    cls = AA.reshape(B, M, A_DIM).astype(np.int64)

    bb, mm, aa = np.meshgrid(np.arange(B), np.arange(M), np.arange(A_DIM),
                             indexing="ij")
    v = hm.ravel()
    bbv, mmv, aav = bb.ravel()[v], mm.ravel()[v], aa.ravel()[v]
    clv = cls.ravel()[v]

    # the class -> (partition, j-slot) map is free (host un-permutes seg);
    # pick the candidate position permutation minimizing the phase-1 slab
    # count S = max same-destination-partition multiplicity per (b, m) row.
    row_key = bbv * M + mmv
    best = None
    rng = np.random.default_rng(12345)
    for trial in range(32):
        pos = (np.arange(C_DIM) if trial == 0
               else rng.permutation(C_DIM)).astype(np.int64)
        ps_t = pos[clv] % 128
        key = row_key * 128 + ps_t
        _, counts = np.unique(key, return_counts=True)
        s_max = int(counts.max())
        if best is None or s_max < best[0]:
            best = (s_max, pos)
        if best[0] <= 6:
            break
    pos = best[1]
    psv = pos[clv] % 128
    jcv = pos[clv] // 128

    def group_rank(keys):
        # rank of each element within its key group, preserving input order
        order = np.argsort(keys, kind="stable")
        sk = keys[order]
        starts = np.r_[0, np.flatnonzero(sk[1:] != sk[:-1]) + 1]
        grp = np.zeros(len(sk), dtype=np.int64)
        grp[starts] = 1
        gid = np.cumsum(grp) - 1
        rank_sorted = np.arange(len(sk)) - starts[gid]
        ranks = np.empty(len(sk), dtype=np.int64)
        ranks[order] = rank_sorted
        return ranks

    s_slot = group_rank((bbv * M + mmv) * 128 + psv)
    k_slot = group_rank(bbv * C_DIM + clv)
    S = int(s_slot.max()) + 1
    K = int(k_slot.max()) + 1
    assert 128 * S <= 2046 and 16 * K <= 2046, (S, K)

    ph1_idx = np.full((B, M, A_DIM), -1, dtype=np.int16)
    ph1_idx[bbv, mmv, aav] = (psv * S + s_slot).astype(np.int16)
    ph3_idx = np.full((B, 128, 128 * S), -1, dtype=np.int16)
    ph3_idx[bbv, psv, s_slot * 128 + mmv] = (jcv * K + k_slot).astype(np.int16)
    return ph1_idx, ph3_idx, S, K, pos


def _build_program(S: int, K: int):
    import concourse.mybir as mybir
    import concourse.tile as tile
    from concourse import bacc
    from concourse.masks import make_identity

    W1 = 128 * S
    W3 = 16 * K
    W1H = (S + 1) // 2 * 128      # first-half width of the transpose psum
    f32, f16, i16 = mybir.dt.float32, mybir.dt.float16, mybir.dt.int16

    nc = bacc.Bacc("TRN2", target_bir_lowering=False, debug=False)
    inpT_d = nc.dram_tensor("inpT", [128, 4, KI, 256], f32, kind="ExternalInput")
    w_aT_d = nc.dram_tensor("w_aT", [128, KI, A_DIM], f32, kind="ExternalInput")
    w_o16_d = nc.dram_tensor("w_o16", [128, KI, O_DIM], f16, kind="ExternalInput")
    b_oa_d = nc.dram_tensor("b_oa", [1, OA], f32, kind="ExternalInput")
    pooledT_d = nc.dram_tensor("pooledT", [128, KI, B], f16, kind="ExternalInput")
    w_neT_d = nc.dram_tensor("w_neT", [128, KI, CSH], f16, kind="ExternalInput")
    b_ne_d = nc.dram_tensor("b_ne", [1, CSH], f16, kind="ExternalInput")
    ph1_d = nc.dram_tensor("ph1", [128, BPC, A_DIM], i16, kind="ExternalInput")
    ph3_d = nc.dram_tensor("ph3", [128, BPC, W1], i16, kind="ExternalInput")
    obj_d = nc.dram_tensor("obj_out", [BPC, 128, O_DIM], f32, kind="ExternalOutput")
    act_d = nc.dram_tensor("act_out", [BPC, 128, A_DIM], f32, kind="ExternalOutput")
    seg_d = nc.dram_tensor("seg_out", [128, BPC, 16], f32, kind="ExternalOutput")
    ne_d = nc.dram_tensor("ne_out", [B, CSH], f32, kind="ExternalOutput")

    with tile.TileContext(nc) as tc, ExitStack() as ctx:
        const = ctx.enter_context(tc.tile_pool(name="const", bufs=1))
        work = ctx.enter_context(tc.tile_pool(name="work", bufs=5))
        ps_ac = ctx.enter_context(tc.tile_pool(name="ps_ac", bufs=3, space="PSUM"))
        ps_ob = ctx.enter_context(tc.tile_pool(name="ps_ob", bufs=1, space="PSUM"))
        ps_tp = ctx.enter_context(tc.tile_pool(name="ps_tp", bufs=2, space="PSUM"))
        ps_ne = ctx.enter_context(tc.tile_pool(name="ps_ne", bufs=1, space="PSUM"))

        ident16 = const.tile([128, 128], f16)
        make_identity(nc, ident16[:])
        ones16 = const.tile([1, B], f16)
        nc.vector.memset(ones16[:], 1.0)
        ones_row = const.tile([1, 128], f32)
        nc.vector.memset(ones_row[:], 1.0)

        # ---- resident inputs (few multi-level-AP DMAs, critical-first) ----
        # order matters: the HWDGE queue is FIFO, so load what batch 0's
        # GEMM + scatter need first (w_oa halves, inpT stripe 0, ph1).
        w_a_sb = const.tile([128, KI, A_DIM], f32)
        w_o16_sb = const.tile([128, KI, O_DIM], f16)
        b_oa_sb = const.tile([1, OA], f32)
        inpT_sb = const.tile([128, 4, KI, 256], f32)
        x16_sb = const.tile([128, 4, KI, 256], f16)
        ph1_sb = const.tile([128, BPC, A_DIM], i16)
        pooledT_sb = const.tile([128, KI, B], f16)
        w_ne_sb = const.tile([128, KI, CSH], f16)
        b_ne_sb = const.tile([1, CSH], f16)

        nc.sync.dma_start(w_a_sb[:], w_aT_d[:])
        nc.sync.dma_start(b_oa_sb[:], b_oa_d[:])
        nc.sync.dma_start(inpT_sb[:, 0], inpT_d[:, 0])
        nc.sync.dma_start(ph1_sb[:], ph1_d[:])
        ph3_sb = const.tile([128, BPC, W1], i16)
        nc.sync.dma_start(ph3_sb[:, :2, :], ph3_d[:, :2, :])
        nc.sync.dma_start(w_o16_sb[:], w_o16_d[:])
        nc.sync.dma_start(inpT_sb[:, 1], inpT_d[:, 1])
        nc.sync.dma_start(ph3_sb[:, 2:5, :], ph3_d[:, 2:5, :])
        nc.sync.dma_start(inpT_sb[:, 2], inpT_d[:, 2])
        nc.sync.dma_start(inpT_sb[:, 3], inpT_d[:, 3])
        nc.sync.dma_start(ph3_sb[:, 5:, :], ph3_d[:, 5:, :])
        nc.sync.dma_start(pooledT_sb[:], pooledT_d[:])
        nc.sync.dma_start(w_ne_sb[:], w_neT_d[:])
        nc.sync.dma_start(b_ne_sb[:], b_ne_d[:])

        # bias broadcast row -> [128, OA] tile (built once via PE)
        bias_ps = ps_ne.tile([128, OA], f32, tag="bias")
        nc.tensor.matmul(bias_ps[:], ones_row[:1, :], b_oa_sb[:],
                         start=True, stop=True)
        bias_bc = const.tile([128, OA], f32)
        nc.scalar.copy(bias_bc[:], bias_ps[:])

        seg_all = const.tile([128, BPC, 16], f32)

        # ---- per-batch pipeline, software-staged so each in-order engine
        # never stalls on a peer: stage A = GEMM + evac + phase-1 scatter,
        # stage B = slab transposes (PE) + evac, stage C = phase-3 scatter +
        # reduce. B lags A by 2 batches, C lags B by 1.
        t1s, t2s = {}, {}

        def stage_a(b):
            g, h = b // 2, b % 2
            tcols = slice(h * 128, (h + 1) * 128)
            act_ps = ps_ac.tile([128, A_DIM], f32, tag="ac")
            for ki in range(KI):
                nc.tensor.matmul(act_ps[:], inpT_sb[:, g, ki, tcols],
                                 w_a_sb[:, ki, :],
                                 start=(ki == 0), stop=(ki == KI - 1))
            act_sb = work.tile([128, A_DIM], f32)
            nc.vector.tensor_add(act_sb[:], act_ps[:], bias_bc[:, O_DIM:])
            nc.sync.dma_start(act_d[b], act_sb[:])
            act16 = work.tile([128, A_DIM], f16)
            nc.scalar.copy(act16[:], act_sb[:])
            t1 = work.tile([128, W1], f16, tag="t1")
            nc.gpsimd.local_scatter(t1[:], act16[:], ph1_sb[:, b, :],
                                    channels=128, num_elems=W1, num_idxs=A_DIM)
            t1s[b] = t1

        def stage_b(b):
            # slab transposes: t2[p, s*128+m] = t1[m, p*S+s]
            t1v = t1s.pop(b)[:].rearrange("p (q s) -> p s q", s=S)
            t2 = work.tile([128, W1], f16, tag="t2")
            for lo, hi in ((0, (S + 1) // 2), ((S + 1) // 2, S)):
                if lo >= hi:
                    continue
                tp = ps_tp.tile([128, W1H], f16, tag="tp")
                for s in range(lo, hi):
                    o = (s - lo) * 128
                    nc.tensor.transpose(tp[:, o:o + 128], t1v[:, s, :], ident16[:])
                nc.vector.tensor_copy(t2[:, lo * 128:hi * 128],
                                      tp[:, :(hi - lo) * 128])
            t2s[b] = t2

        def stage_c(b):
            t3 = work.tile([128, W3], f16, tag="t3")
            nc.gpsimd.local_scatter(t3[:], t2s.pop(b)[:], ph3_sb[:, b, :],
                                    channels=128, num_elems=W3, num_idxs=W1)
            nc.vector.tensor_reduce(out=seg_all[:, b, :],
                                    in_=t3[:].rearrange("p (j k) -> p j k", k=K),
                                    axis=mybir.AxisListType.X,
                                    op=mybir.AluOpType.max)

        def stage_d(b):
            g, h = b // 2, b % 2
            tcols = slice(h * 128, (h + 1) * 128)
            obj_ps = ps_ob.tile([128, O_DIM], f32, tag="ob")
            for ki in range(KI):
                nc.tensor.matmul(obj_ps[:], x16_sb[:, g, ki, tcols],
                                 w_o16_sb[:, ki, :],
                                 start=(ki == 0), stop=(ki == KI - 1))
            obj_sb = work.tile([128, O_DIM], f32, tag="obj_sb")
            nc.vector.tensor_add(obj_sb[:], obj_ps[:], bias_bc[:, :O_DIM])
            nc.sync.dma_start(obj_d[b], obj_sb[:])

        for b in range(BPC + 2):
            if b < BPC:
                stage_a(b)
            if b % 2 == 1 and b // 2 < 4:
                g = b // 2
                nc.scalar.copy(x16_sb[:, g], inpT_sb[:, g])
            if 1 <= b < BPC + 1:
                stage_b(b - 1)
            if b >= 2:
                stage_c(b - 2)
            if 2 <= b < BPC + 2:
                stage_d(b - 2)
            if b == BPC:
                # non-exist head, class-sharded: ne[all 64 b, 256 shard].
                # Emitted late so its PE chain fills the pipeline drain (its
                # weights also arrive last in the DMA queue).
                ne_ps = ps_ne.tile([B, CSH], f32)
                nc.tensor.matmul(ne_ps[:], ones16[:], b_ne_sb[:],
                                 start=True, stop=False)
                for ki in range(KI):
                    nc.tensor.matmul(ne_ps[:], pooledT_sb[:, ki, :],
                                     w_ne_sb[:, ki, :],
                                     start=False, stop=(ki == KI - 1))
                ne_sb = work.tile([B, CSH], f32)
                nc.scalar.copy(ne_sb[:], ne_ps[:])
                nc.sync.dma_start(ne_d[:], ne_sb[:])
        nc.sync.dma_start(seg_d[:], seg_all[:])

    nc.compile()
    return nc


def _prep_host(inputs):
    inp = np.ascontiguousarray(inputs["inp"], dtype=np.float32)
    objmask = np.asarray(inputs["objmask"], dtype=np.float32)
    AA = np.asarray(inputs["AAidxs_tgts"]).astype(np.int64)
    W_obj = np.asarray(inputs["W_obj"], dtype=np.float32)
    b_obj = np.asarray(inputs["b_obj"], dtype=np.float32)
    W_act = np.asarray(inputs["W_act"], dtype=np.float32)
    b_act = np.asarray(inputs["b_act"], dtype=np.float32)
    W_ne = np.asarray(inputs["W_ne"], dtype=np.float32)
    b_ne = np.asarray(inputs["b_ne"], dtype=np.float32)

    ph1_idx, ph3_idx, S, K, pos = _build_routing(objmask, AA)

    # weights, chunked [KI, 128, cols]
    w_aT = np.ascontiguousarray(
        W_act.T.reshape(KI, 128, A_DIM).transpose(1, 0, 2))
    w_o16 = np.ascontiguousarray(
        W_obj.T.astype(np.float16).reshape(KI, 128, O_DIM).transpose(1, 0, 2))
    b_oa = np.concatenate([b_obj, b_act])[None, :]

    # normalized-mask pooling on host (pure input function)
    nmask = objmask / objmask.sum(axis=1, keepdims=True)
    pooled = np.einsum("bmd,bm->bd", inp, nmask).astype(np.float32)
    pooledT = np.ascontiguousarray(
        pooled.T.reshape(KI, 128, B).transpose(1, 0, 2)).astype(np.float16)

    in_maps = []
    for c in range(NCORES):
        bs = slice(c * BPC, (c + 1) * BPC)
        cs = slice(c * (C_DIM // NCORES), (c + 1) * (C_DIM // NCORES))
        inpT = inp[bs].reshape(BPC * 128, D).T.reshape(KI, 128, 4, 256)
        inpT = np.ascontiguousarray(inpT.transpose(1, 2, 0, 3))
        w_neT = np.zeros((128, KI, CSH), np.float16)
        w_neT[:, :, :C_DIM // NCORES] = W_ne[cs].T.astype(np.float16).reshape(
            KI, 128, C_DIM // NCORES).transpose(1, 0, 2)
        b_ne_p = np.zeros((1, CSH), np.float16)
        b_ne_p[0, :C_DIM // NCORES] = b_ne[cs].astype(np.float16)
        in_maps.append({
            "inpT": inpT,
            "w_aT": w_aT,
            "w_o16": w_o16,
            "b_oa": b_oa,
            "pooledT": pooledT,
            "w_neT": w_neT,
            "b_ne": b_ne_p,
            "ph1": np.ascontiguousarray(ph1_idx[bs].transpose(1, 0, 2)),
            "ph3": np.ascontiguousarray(ph3_idx[bs].transpose(1, 0, 2)),
        })
    return in_maps, S, K, pos


def _assemble(outs, pos):
    """Combine per-core results into full outputs (+ host where-combine)."""
    obj_out = np.concatenate([np.asarray(o["obj_out"]) for o in outs])
    act_out = np.concatenate([np.asarray(o["act_out"]) for o in outs])
    # seg_out[p, b, j] holds the class at permuted position j*128 + p
    seg_pos = np.concatenate(
        [np.transpose(np.asarray(o["seg_out"]), (1, 2, 0)).reshape(BPC, 2048)
         for o in outs])
    seg = seg_pos[:, pos]
    ne = np.concatenate(
        [np.asarray(o["ne_out"])[:, :C_DIM // NCORES] for o in outs], axis=1)
    final = np.where(seg > 0, seg, ne).astype(np.float32)
    return final, act_out, obj_out


def kernel(**inputs) -> tuple:
    from concourse.bass_utils import run_bass_kernel_spmd

    in_maps, S, K, pos = _prep_host(inputs)
    key = (S, K)
    if key not in _prog_cache:
        _prog_cache[key] = _build_program(S, K)
    nc = _prog_cache[key]

    res = run_bass_kernel_spmd(nc, in_maps, core_ids=list(range(NCORES)))
    return _assemble(res.results, pos)


# revision 57
# speedup vs baseline: 1.0878x; 1.0073x over previous
"""Trainium2 Bass kernel for nn_EDM_task1 (scatter_memory).

Self-contained: takes FULL inputs, shards batch-parallel over 8 NeuronCores,
runs one SPMD Bass/Tile program per core, gathers FULL outputs.

Per batch row b the reference computes:
  obj_out = inp @ W_obj.T + b_obj                       [B, 128, 200]
  act_out = inp @ W_act.T + b_act                       [B, 128, 100]
  ne_out  = masked-mean-pool(inp) @ W_ne.T + b_ne       [B, 2000]
  scatter-max of selected act logits into classes; final = where(touched
  and positive, per-class max of positive logits, ne_out).

Mathematically (validated vs reference): with hostmask = objmask & (tgt!=-1),
  seg[b,c] = max(0, max_{n: hostmask, tgt=c} act_flat[b,n])
  final[b,c] = seg > 0 ? seg : ne_out      (selected logits are all > 0)

Device work split:
  - batches are data-parallel over cores (inp, act/obj GEMM, scatter-max)
  - W_ne is class-sharded over cores (each computes ne for ALL batches over
    its 250 classes); the trivial elementwise where() combine runs on host.

The scatter-max runs on-device as: per-partition GPSIMD local_scatter (by
destination partition) -> PE slab transposes -> per-partition local_scatter
(by class slot) -> DVE max-reduce over slots. Routing tables are pure
functions of integer inputs and are precomputed on host.
"""
from contextlib import ExitStack

import numpy as np

B, M, D = 64, 128, 2048
A_DIM, O_DIM, C_DIM = 100, 200, 2000
OA = O_DIM + A_DIM          # 300
NCORES = 8
BPC = B // NCORES           # 8 batches per core
KI = D // 128               # 16 contraction tiles
CSH = 256                   # padded ne class shard per core (250 real)

_prog_cache: dict = {}


def _build_routing(objmask: np.ndarray, AA: np.ndarray):
    """Vectorized host-side routing tables.

    ph1_idx [B,128,100] i16: per-(b,m) scatter dest = pstar*S + s, or -1.
    ph3_idx [B,128,128*S] i16: per-(b,pstar) dest = j*K + k at transposed
        position s*128 + m, or -1.  (class c = j*128 + pstar)
    """
    hm = ((objmask[:, :, None] > 0) & (AA.reshape(B, M, A_DIM) != -1))
    cls = AA.reshape(B, M, A_DIM).astype(np.int64)

    bb, mm, aa = np.meshgrid(np.arange(B), np.arange(M), np.arange(A_DIM),
                             indexing="ij")
    v = hm.ravel()
    bbv, mmv, aav = bb.ravel()[v], mm.ravel()[v], aa.ravel()[v]
    clv = cls.ravel()[v]

    # the class -> (partition, j-slot) map is free (host un-permutes seg);
    # pick the candidate position permutation minimizing the phase-1 slab
    # count S = max same-destination-partition multiplicity per (b, m) row.
    row_key = bbv * M + mmv
    best = None
    rng = np.random.default_rng(12345)
    for trial in range(32):
        pos = (np.arange(C_DIM) if trial == 0
               else rng.permutation(C_DIM)).astype(np.int64)
        ps_t = pos[clv] % 128
        key = row_key * 128 + ps_t
        _, counts = np.unique(key, return_counts=True)
        s_max = int(counts.max())
        if best is None or s_max < best[0]:
            best = (s_max, pos)
        if best[0] <= 6:
            break
    pos = best[1]
    psv = pos[clv] % 128
    jcv = pos[clv] // 128

    def group_rank(keys):
        # rank of each element within its key group, preserving input order
        order = np.argsort(keys, kind="stable")
        sk = keys[order]
        starts = np.r_[0, np.flatnonzero(sk[1:] != sk[:-1]) + 1]
        grp = np.zeros(len(sk), dtype=np.int64)
        grp[starts] = 1
        gid = np.cumsum(grp) - 1
        rank_sorted = np.arange(len(sk)) - starts[gid]
        ranks = np.empty(len(sk), dtype=np.int64)
        ranks[order] = rank_sorted
        return ranks

    s_slot = group_rank((bbv * M + mmv) * 128 + psv)
    k_slot = group_rank(bbv * C_DIM + clv)
    S = int(s_slot.max()) + 1
    K = int(k_slot.max()) + 1
    assert 128 * S <= 2046 and 16 * K <= 2046, (S, K)

    ph1_idx = np.full((B, M, A_DIM), -1, dtype=np.int16)
    ph1_idx[bbv, mmv, aav] = (psv * S + s_slot).astype(np.int16)
    ph3_idx = np.full((B, 128, 128 * S), -1, dtype=np.int16)
    ph3_idx[bbv, psv, s_slot * 128 + mmv] = (jcv * K + k_slot).astype(np.int16)
    return ph1_idx, ph3_idx, S, K, pos


def _build_program(S: int, K: int):
    import concourse.mybir as mybir
    import concourse.tile as tile
    from concourse import bacc
    from concourse.masks import make_identity

    W1 = 128 * S
    W3 = 16 * K
    W1H = (S + 1) // 2 * 128      # first-half width of the transpose psum
    f32, f16, i16 = mybir.dt.float32, mybir.dt.float16, mybir.dt.int16

    nc = bacc.Bacc("TRN2", target_bir_lowering=False, debug=False)
    inpT_d = nc.dram_tensor("inpT", [128, 4, KI, 256], f32, kind="ExternalInput")
    w_aT_d = nc.dram_tensor("w_aT", [128, KI, A_DIM], f32, kind="ExternalInput")
    w_o16_d = nc.dram_tensor("w_o16", [128, KI, O_DIM], f16, kind="ExternalInput")
    b_oa_d = nc.dram_tensor("b_oa", [1, OA], f32, kind="ExternalInput")
    pooledT_d = nc.dram_tensor("pooledT", [128, KI, B], f16, kind="ExternalInput")
    w_neT_d = nc.dram_tensor("w_neT", [128, KI, CSH], f16, kind="ExternalInput")
    b_ne_d = nc.dram_tensor("b_ne", [1, CSH], f16, kind="ExternalInput")
    ph1_d = nc.dram_tensor("ph1", [128, BPC, A_DIM], i16, kind="ExternalInput")
    ph3_d = nc.dram_tensor("ph3", [128, BPC, W1], i16, kind="ExternalInput")
    obj_d = nc.dram_tensor("obj_out", [BPC, 128, O_DIM], f32, kind="ExternalOutput")
    act_d = nc.dram_tensor("act_out", [BPC, 128, A_DIM], f32, kind="ExternalOutput")
    seg_d = nc.dram_tensor("seg_out", [128, BPC, 16], f32, kind="ExternalOutput")
    ne_d = nc.dram_tensor("ne_out", [B, CSH], f32, kind="ExternalOutput")

    with tile.TileContext(nc) as tc, ExitStack() as ctx:
        const = ctx.enter_context(tc.tile_pool(name="const", bufs=1))
        work = ctx.enter_context(tc.tile_pool(name="work", bufs=5))
        ps_ac = ctx.enter_context(tc.tile_pool(name="ps_ac", bufs=3, space="PSUM"))
        ps_ob = ctx.enter_context(tc.tile_pool(name="ps_ob", bufs=1, space="PSUM"))
        ps_tp = ctx.enter_context(tc.tile_pool(name="ps_tp", bufs=2, space="PSUM"))
        ps_ne = ctx.enter_context(tc.tile_pool(name="ps_ne", bufs=1, space="PSUM"))

        ident16 = const.tile([128, 128], f16)
        make_identity(nc, ident16[:])
        ones16 = const.tile([1, B], f16)
        nc.vector.memset(ones16[:], 1.0)
        ones_row = const.tile([1, 128], f32)
        nc.vector.memset(ones_row[:], 1.0)

        # ---- resident inputs (few multi-level-AP DMAs, critical-first) ----
        # order matters: the HWDGE queue is FIFO, so load what batch 0's
        # GEMM + scatter need first (w_oa halves, inpT stripe 0, ph1).
        w_a_sb = const.tile([128, KI, A_DIM], f32)
        w_o16_sb = const.tile([128, KI, O_DIM], f16)
        b_oa_sb = const.tile([1, OA], f32)
        inpT_sb = const.tile([128, 4, KI, 256], f32)
        x16_sb = const.tile([128, 4, KI, 256], f16)
        ph1_sb = const.tile([128, BPC, A_DIM], i16)
        pooledT_sb = const.tile([128, KI, B], f16)
        w_ne_sb = const.tile([128, KI, CSH], f16)
        b_ne_sb = const.tile([1, CSH], f16)

        nc.sync.dma_start(w_a_sb[:], w_aT_d[:])
        nc.sync.dma_start(b_oa_sb[:], b_oa_d[:])
        nc.sync.dma_start(inpT_sb[:, 0], inpT_d[:, 0])
        nc.sync.dma_start(ph1_sb[:], ph1_d[:])
        ph3_sb = const.tile([128, BPC, W1], i16)
        nc.sync.dma_start(ph3_sb[:, :2, :], ph3_d[:, :2, :])
        nc.sync.dma_start(w_o16_sb[:], w_o16_d[:])
        nc.sync.dma_start(inpT_sb[:, 1], inpT_d[:, 1])
        nc.sync.dma_start(ph3_sb[:, 2:5, :], ph3_d[:, 2:5, :])
        nc.sync.dma_start(inpT_sb[:, 2], inpT_d[:, 2])
        nc.sync.dma_start(inpT_sb[:, 3], inpT_d[:, 3])
        nc.sync.dma_start(ph3_sb[:, 5:, :], ph3_d[:, 5:, :])
        nc.sync.dma_start(pooledT_sb[:], pooledT_d[:])
        nc.sync.dma_start(w_ne_sb[:], w_neT_d[:])
        nc.sync.dma_start(b_ne_sb[:], b_ne_d[:])

        # bias broadcast row -> [128, OA] tile (built once via PE)
        bias_ps = ps_ne.tile([128, OA], f32, tag="bias")
        nc.tensor.matmul(bias_ps[:], ones_row[:1, :], b_oa_sb[:],
                         start=True, stop=True)
        bias_bc = const.tile([128, OA], f32)
        nc.scalar.copy(bias_bc[:], bias_ps[:])

        seg_all = const.tile([128, BPC, 16], f32)

        # ---- per-batch pipeline, software-staged so each in-order engine
        # never stalls on a peer: stage A = GEMM + evac + phase-1 scatter,
        # stage B = slab transposes (PE) + evac, stage C = phase-3 scatter +
        # reduce. B lags A by 2 batches, C lags B by 1.
        t1s, t2s = {}, {}

        def stage_a(b):
            g, h = b // 2, b % 2
            tcols = slice(h * 128, (h + 1) * 128)
            act_ps = ps_ac.tile([128, A_DIM], f32, tag="ac")
            for ki in range(KI):
                nc.tensor.matmul(act_ps[:], inpT_sb[:, g, ki, tcols],
                                 w_a_sb[:, ki, :],
                                 start=(ki == 0), stop=(ki == KI - 1))
            act_sb = work.tile([128, A_DIM], f32)
            nc.vector.tensor_add(act_sb[:], act_ps[:], bias_bc[:, O_DIM:])
            nc.sync.dma_start(act_d[b], act_sb[:])
            act16 = work.tile([128, A_DIM], f16)
            nc.scalar.copy(act16[:], act_sb[:])
            t1 = work.tile([128, W1], f16, tag="t1")
            nc.gpsimd.local_scatter(t1[:], act16[:], ph1_sb[:, b, :],
                                    channels=128, num_elems=W1, num_idxs=A_DIM)
            t1s[b] = t1

        def stage_b(b):
            # slab transposes: t2[p, s*128+m] = t1[m, p*S+s]
            t1v = t1s.pop(b)[:].rearrange("p (q s) -> p s q", s=S)
            t2 = work.tile([128, W1], f16, tag="t2")
            for lo, hi in ((0, (S + 1) // 2), ((S + 1) // 2, S)):
                if lo >= hi:
                    continue
                tp = ps_tp.tile([128, W1H], f16, tag="tp")
                for s in range(lo, hi):
                    o = (s - lo) * 128
                    nc.tensor.transpose(tp[:, o:o + 128], t1v[:, s, :], ident16[:])
                nc.vector.tensor_copy(t2[:, lo * 128:hi * 128],
                                      tp[:, :(hi - lo) * 128])
            t2s[b] = t2

        def stage_c(b):
            t3 = work.tile([128, W3], f16, tag="t3")
            nc.gpsimd.local_scatter(t3[:], t2s.pop(b)[:], ph3_sb[:, b, :],
                                    channels=128, num_elems=W3, num_idxs=W1)
            nc.vector.tensor_reduce(out=seg_all[:, b, :],
                                    in_=t3[:].rearrange("p (j k) -> p j k", k=K),
                                    axis=mybir.AxisListType.X,
                                    op=mybir.AluOpType.max)

        def stage_d(b):
            g, h = b // 2, b % 2
            tcols = slice(h * 128, (h + 1) * 128)
            obj_ps = ps_ob.tile([128, O_DIM], f32, tag="ob")
            for ki in range(KI):
                nc.tensor.matmul(obj_ps[:], x16_sb[:, g, ki, tcols],
                                 w_o16_sb[:, ki, :],
                                 start=(ki == 0), stop=(ki == KI - 1))
            obj_sb = work.tile([128, O_DIM], f32, tag="obj_sb")
            nc.vector.tensor_add(obj_sb[:], obj_ps[:], bias_bc[:, :O_DIM])
            nc.sync.dma_start(obj_d[b], obj_sb[:])

        for b in range(BPC + 2):
            if b < BPC:
                stage_a(b)
            if b % 2 == 1 and b // 2 < 4:
                g = b // 2
                nc.scalar.copy(x16_sb[:, g], inpT_sb[:, g])
            if 1 <= b < BPC + 1:
                stage_b(b - 1)
            if b >= 2:
                stage_c(b - 2)
            if 2 <= b < BPC + 2:
                stage_d(b - 2)
            if b == BPC:
                # non-exist head, class-sharded: ne[all 64 b, 256 shard].
                # Emitted late so its PE chain fills the pipeline drain (its
                # weights also arrive last in the DMA queue).
                ne_ps = ps_ne.tile([B, CSH], f32)
                nc.tensor.matmul(ne_ps[:], ones16[:], b_ne_sb[:],
                                 start=True, stop=False)
                for ki in range(KI):
                    nc.tensor.matmul(ne_ps[:], pooledT_sb[:, ki, :],
                                     w_ne_sb[:, ki, :],
                                     start=False, stop=(ki == KI - 1))
                ne_sb = work.tile([B, CSH], f32)
                nc.scalar.copy(ne_sb[:], ne_ps[:])
                nc.sync.dma_start(ne_d[:], ne_sb[:])
        nc.sync.dma_start(seg_d[:], seg_all[:])

    nc.compile()
    return nc


def _prep_host(inputs):
    inp = np.ascontiguousarray(inputs["inp"], dtype=np.float32)
    objmask = np.asarray(inputs["objmask"], dtype=np.float32)
    AA = np.asarray(inputs["AAidxs_tgts"]).astype(np.int64)
    W_obj = np.asarray(inputs["W_obj"], dtype=np.float32)
    b_obj = np.asarray(inputs["b_obj"], dtype=np.float32)
    W_act = np.asarray(inputs["W_act"], dtype=np.float32)
    b_act = np.asarray(inputs["b_act"], dtype=np.float32)
    W_ne = np.asarray(inputs["W_ne"], dtype=np.float32)
    b_ne = np.asarray(inputs["b_ne"], dtype=np.float32)

    ph1_idx, ph3_idx, S, K, pos = _build_routing(objmask, AA)

    # weights, chunked [KI, 128, cols]
    w_aT = np.ascontiguousarray(
        W_act.T.reshape(KI, 128, A_DIM).transpose(1, 0, 2))
    w_o16 = np.ascontiguousarray(
        W_obj.T.astype(np.float16).reshape(KI, 128, O_DIM).transpose(1, 0, 2))
    b_oa = np.concatenate([b_obj, b_act])[None, :]

    # normalized-mask pooling on host (pure input function)
    nmask = objmask / objmask.sum(axis=1, keepdims=True)
    pooled = np.einsum("bmd,bm->bd", inp, nmask).astype(np.float32)
    pooledT = np.ascontiguousarray(
        pooled.T.reshape(KI, 128, B).transpose(1, 0, 2)).astype(np.float16)

    in_maps = []
    for c in range(NCORES):
        bs = slice(c * BPC, (c + 1) * BPC)
        cs = slice(c * (C_DIM // NCORES), (c + 1) * (C_DIM // NCORES))
        inpT = inp[bs].reshape(BPC * 128, D).T.reshape(KI, 128, 4, 256)
        inpT = np.ascontiguousarray(inpT.transpose(1, 2, 0, 3))
        w_neT = np.zeros((128, KI, CSH), np.float16)
        w_neT[:, :, :C_DIM // NCORES] = W_ne[cs].T.astype(np.float16).reshape(
            KI, 128, C_DIM // NCORES).transpose(1, 0, 2)
        b_ne_p = np.zeros((1, CSH), np.float16)
        b_ne_p[0, :C_DIM // NCORES] = b_ne[cs].astype(np.float16)
        in_maps.append({
            "inpT": inpT,
            "w_aT": w_aT,
            "w_o16": w_o16,
            "b_oa": b_oa,
            "pooledT": pooledT,
            "w_neT": w_neT,
            "b_ne": b_ne_p,
            "ph1": np.ascontiguousarray(ph1_idx[bs].transpose(1, 0, 2)),
            "ph3": np.ascontiguousarray(ph3_idx[bs].transpose(1, 0, 2)),
        })
    return in_maps, S, K, pos


def _assemble(outs, pos):
    """Combine per-core results into full outputs (+ host where-combine)."""
    obj_out = np.concatenate([np.asarray(o["obj_out"]) for o in outs])
    act_out = np.concatenate([np.asarray(o["act_out"]) for o in outs])
    # seg_out[p, b, j] holds the class at permuted position j*128 + p
    seg_pos = np.concatenate(
        [np.transpose(np.asarray(o["seg_out"]), (1, 2, 0)).reshape(BPC, 2048)
         for o in outs])
    seg = seg_pos[:, pos]
    ne = np.concatenate(
        [np.asarray(o["ne_out"])[:, :C_DIM // NCORES] for o in outs], axis=1)
    final = np.where(seg > 0, seg, ne).astype(np.float32)
    return final, act_out, obj_out


def kernel(**inputs) -> tuple:
    from concourse.bass_utils import run_bass_kernel_spmd

    in_maps, S, K, pos = _prep_host(inputs)
    key = (S, K)
    if key not in _prog_cache:
        _prog_cache[key] = _build_program(S, K)
    nc = _prog_cache[key]

    res = run_bass_kernel_spmd(nc, in_maps, core_ids=list(range(NCORES)))
    return _assemble(res.results, pos)


# revision 59
# speedup vs baseline: 1.0890x; 1.0011x over previous
"""Trainium2 Bass kernel for nn_EDM_task1 (scatter_memory).

Self-contained: takes FULL inputs, shards batch-parallel over 8 NeuronCores,
runs one SPMD Bass/Tile program per core, gathers FULL outputs.

Per batch row b the reference computes:
  obj_out = inp @ W_obj.T + b_obj                       [B, 128, 200]
  act_out = inp @ W_act.T + b_act                       [B, 128, 100]
  ne_out  = masked-mean-pool(inp) @ W_ne.T + b_ne       [B, 2000]
  scatter-max of selected act logits into classes; final = where(touched
  and positive, per-class max of positive logits, ne_out).

Mathematically (validated vs reference): with hostmask = objmask & (tgt!=-1),
  seg[b,c] = max(0, max_{n: hostmask, tgt=c} act_flat[b,n])
  final[b,c] = seg > 0 ? seg : ne_out      (selected logits are all > 0)

Device work split:
  - batches are data-parallel over cores (inp, act/obj GEMM, scatter-max)
  - W_ne is class-sharded over cores (each computes ne for ALL batches over
    its 250 classes); the trivial elementwise where() combine runs on host.

The scatter-max runs on-device as: per-partition GPSIMD local_scatter (by
destination partition) -> PE slab transposes -> per-partition local_scatter
(by class slot) -> DVE max-reduce over slots. Routing tables are pure
functions of integer inputs and are precomputed on host.
"""
from contextlib import ExitStack

import numpy as np

B, M, D = 64, 128, 2048
A_DIM, O_DIM, C_DIM = 100, 200, 2000
OA = O_DIM + A_DIM          # 300
NCORES = 8
BPC = B // NCORES           # 8 batches per core
KI = D // 128               # 16 contraction tiles
CSH = 256                   # padded ne class shard per core (250 real)

_prog_cache: dict = {}


def _build_routing(objmask: np.ndarray, AA: np.ndarray):
    """Vectorized host-side routing tables.

    ph1_idx [B,128,100] i16: per-(b,m) scatter dest = pstar*S + s, or -1.
    ph3_idx [B,128,128*S] i16: per-(b,pstar) dest = j*K + k at transposed
        position s*128 + m, or -1.  (class c = j*128 + pstar)
    """
    hm = ((objmask[:, :, None] > 0) & (AA.reshape(B, M, A_DIM) != -1))
    cls = AA.reshape(B, M, A_DIM).astype(np.int64)

    bb, mm, aa = np.meshgrid(np.arange(B), np.arange(M), np.arange(A_DIM),
                             indexing="ij")
    v = hm.ravel()
    bbv, mmv, aav = bb.ravel()[v], mm.ravel()[v], aa.ravel()[v]
    clv = cls.ravel()[v]

    # the class -> (partition, j-slot) map is free (host un-permutes seg);
    # pick the candidate position permutation minimizing the phase-1 slab
    # count S = max same-destination-partition multiplicity per (b, m) row.
    row_key = bbv * M + mmv
    best = None
    rng = np.random.default_rng(12345)
    for trial in range(32):
        pos = (np.arange(C_DIM) if trial == 0
               else rng.permutation(C_DIM)).astype(np.int64)
        ps_t = pos[clv] % 128
        key = row_key * 128 + ps_t
        _, counts = np.unique(key, return_counts=True)
        s_max = int(counts.max())
        if best is None or s_max < best[0]:
            best = (s_max, pos)
        if best[0] <= 6:
            break
    pos = best[1]
    psv = pos[clv] % 128
    jcv = pos[clv] // 128

    def group_rank(keys):
        # rank of each element within its key group, preserving input order
        order = np.argsort(keys, kind="stable")
        sk = keys[order]
        starts = np.r_[0, np.flatnonzero(sk[1:] != sk[:-1]) + 1]
        grp = np.zeros(len(sk), dtype=np.int64)
        grp[starts] = 1
        gid = np.cumsum(grp) - 1
        rank_sorted = np.arange(len(sk)) - starts[gid]
        ranks = np.empty(len(sk), dtype=np.int64)
        ranks[order] = rank_sorted
        return ranks

    s_slot = group_rank((bbv * M + mmv) * 128 + psv)
    k_slot = group_rank(bbv * C_DIM + clv)
    S = int(s_slot.max()) + 1
    K = int(k_slot.max()) + 1
    assert 128 * S <= 2046 and 16 * K <= 2046, (S, K)

    ph1_idx = np.full((B, M, A_DIM), -1, dtype=np.int16)
    ph1_idx[bbv, mmv, aav] = (psv * S + s_slot).astype(np.int16)
    ph3_idx = np.full((B, 128, 128 * S), -1, dtype=np.int16)
    ph3_idx[bbv, psv, s_slot * 128 + mmv] = (jcv * K + k_slot).astype(np.int16)
    return ph1_idx, ph3_idx, S, K, pos


def _build_program(S: int, K: int):
    import concourse.mybir as mybir
    import concourse.tile as tile
    from concourse import bacc
    from concourse.masks import make_identity

    W1 = 128 * S
    W3 = 16 * K
    W1H = (S + 1) // 2 * 128      # first-half width of the transpose psum
    f32, f16, i16 = mybir.dt.float32, mybir.dt.float16, mybir.dt.int16

    nc = bacc.Bacc("TRN2", target_bir_lowering=False, debug=False)
    inpT_d = nc.dram_tensor("inpT", [128, 4, KI, 256], f32, kind="ExternalInput")
    w_aT_d = nc.dram_tensor("w_aT", [128, KI, A_DIM], f32, kind="ExternalInput")
    w_o16_d = nc.dram_tensor("w_o16", [128, KI, O_DIM], f16, kind="ExternalInput")
    b_oa_d = nc.dram_tensor("b_oa", [1, OA], f32, kind="ExternalInput")
    pooledT_d = nc.dram_tensor("pooledT", [128, KI, B], f16, kind="ExternalInput")
    w_neT_d = nc.dram_tensor("w_neT", [128, KI, CSH], f16, kind="ExternalInput")
    b_ne_d = nc.dram_tensor("b_ne", [1, CSH], f16, kind="ExternalInput")
    ph1_d = nc.dram_tensor("ph1", [128, BPC, A_DIM], i16, kind="ExternalInput")
    ph3_d = nc.dram_tensor("ph3", [128, BPC, W1], i16, kind="ExternalInput")
    obj_d = nc.dram_tensor("obj_out", [BPC, 128, O_DIM], f32, kind="ExternalOutput")
    act_d = nc.dram_tensor("act_out", [BPC, 128, A_DIM], f32, kind="ExternalOutput")
    seg_d = nc.dram_tensor("seg_out", [128, BPC, 16], f32, kind="ExternalOutput")
    ne_d = nc.dram_tensor("ne_out", [B, CSH], f32, kind="ExternalOutput")

    with tile.TileContext(nc) as tc, ExitStack() as ctx:
        const = ctx.enter_context(tc.tile_pool(name="const", bufs=1))
        work = ctx.enter_context(tc.tile_pool(name="work", bufs=5))
        ps_ac = ctx.enter_context(tc.tile_pool(name="ps_ac", bufs=3, space="PSUM"))
        ps_ob = ctx.enter_context(tc.tile_pool(name="ps_ob", bufs=1, space="PSUM"))
        ps_tp = ctx.enter_context(tc.tile_pool(name="ps_tp", bufs=2, space="PSUM"))
        ps_ne = ctx.enter_context(tc.tile_pool(name="ps_ne", bufs=1, space="PSUM"))

        ident16 = const.tile([128, 128], f16)
        make_identity(nc, ident16[:])
        ones16 = const.tile([1, B], f16)
        nc.vector.memset(ones16[:], 1.0)
        ones_row = const.tile([1, 128], f32)
        nc.vector.memset(ones_row[:], 1.0)

        # ---- resident inputs (few multi-level-AP DMAs, critical-first) ----
        # order matters: the HWDGE queue is FIFO, so load what batch 0's
        # GEMM + scatter need first (w_oa halves, inpT stripe 0, ph1).
        w_a_sb = const.tile([128, KI, A_DIM], f32)
        w_o16_sb = const.tile([128, KI, O_DIM], f16)
        b_oa_sb = const.tile([1, OA], f32)
        inpT_sb = const.tile([128, 4, KI, 256], f32)
        x16_sb = const.tile([128, 4, KI, 256], f16)
        ph1_sb = const.tile([128, BPC, A_DIM], i16)
        pooledT_sb = const.tile([128, KI, B], f16)
        w_ne_sb = const.tile([128, KI, CSH], f16)
        b_ne_sb = const.tile([1, CSH], f16)

        nc.sync.dma_start(w_a_sb[:], w_aT_d[:])
        nc.sync.dma_start(b_oa_sb[:], b_oa_d[:])
        nc.sync.dma_start(inpT_sb[:, 0], inpT_d[:, 0])
        nc.sync.dma_start(ph1_sb[:], ph1_d[:])
        ph3_sb = const.tile([128, BPC, W1], i16)
        nc.sync.dma_start(ph3_sb[:, :2, :], ph3_d[:, :2, :])
        nc.sync.dma_start(w_o16_sb[:], w_o16_d[:])
        nc.sync.dma_start(inpT_sb[:, 1], inpT_d[:, 1])
        nc.sync.dma_start(ph3_sb[:, 2:5, :], ph3_d[:, 2:5, :])
        nc.sync.dma_start(inpT_sb[:, 2], inpT_d[:, 2])
        nc.sync.dma_start(inpT_sb[:, 3], inpT_d[:, 3])
        nc.sync.dma_start(ph3_sb[:, 5:, :], ph3_d[:, 5:, :])
        nc.sync.dma_start(pooledT_sb[:], pooledT_d[:])
        nc.sync.dma_start(w_ne_sb[:], w_neT_d[:])
        nc.sync.dma_start(b_ne_sb[:], b_ne_d[:])

        # bias broadcast row -> [128, OA] tile (built once via PE)
        bias_ps = ps_ne.tile([128, OA], f32, tag="bias")
        nc.tensor.matmul(bias_ps[:], ones_row[:1, :], b_oa_sb[:],
                         start=True, stop=True)
        bias_bc = const.tile([128, OA], f32)
        nc.scalar.copy(bias_bc[:], bias_ps[:])

        seg_all = const.tile([128, BPC, 16], f32)

        # ---- per-batch pipeline, software-staged so each in-order engine
        # never stalls on a peer: stage A = GEMM + evac + phase-1 scatter,
        # stage B = slab transposes (PE) + evac, stage C = phase-3 scatter +
        # reduce. B lags A by 2 batches, C lags B by 1.
        t1s, t2s = {}, {}

        def stage_a(b):
            g, h = b // 2, b % 2
            tcols = slice(h * 128, (h + 1) * 128)
            act_ps = ps_ac.tile([128, A_DIM], f32, tag="ac")
            for ki in range(KI):
                nc.tensor.matmul(act_ps[:], inpT_sb[:, g, ki, tcols],
                                 w_a_sb[:, ki, :],
                                 start=(ki == 0), stop=(ki == KI - 1))
            act_sb = work.tile([128, A_DIM], f32)
            nc.vector.tensor_add(act_sb[:], act_ps[:], bias_bc[:, O_DIM:])
            nc.sync.dma_start(act_d[b], act_sb[:])
            act16 = work.tile([128, A_DIM], f16)
            nc.scalar.copy(act16[:], act_sb[:])
            t1 = work.tile([128, W1], f16, tag="t1")
            nc.gpsimd.local_scatter(t1[:], act16[:], ph1_sb[:, b, :],
                                    channels=128, num_elems=W1, num_idxs=A_DIM)
            t1s[b] = t1

        def stage_b(b):
            # slab transposes: t2[p, s*128+m] = t1[m, p*S+s]
            t1v = t1s.pop(b)[:].rearrange("p (q s) -> p s q", s=S)
            t2 = work.tile([128, W1], f16, tag="t2")
            for lo, hi in ((0, (S + 1) // 2), ((S + 1) // 2, S)):
                if lo >= hi:
                    continue
                tp = ps_tp.tile([128, W1H], f16, tag="tp")
                for s in range(lo, hi):
                    o = (s - lo) * 128
                    nc.tensor.transpose(tp[:, o:o + 128], t1v[:, s, :], ident16[:])
                nc.vector.tensor_copy(t2[:, lo * 128:hi * 128],
                                      tp[:, :(hi - lo) * 128])
            t2s[b] = t2

        def stage_c(b):
            t3 = work.tile([128, W3], f16, tag="t3")
            nc.gpsimd.local_scatter(t3[:], t2s.pop(b)[:], ph3_sb[:, b, :],
                                    channels=128, num_elems=W3, num_idxs=W1)
            nc.vector.tensor_reduce(out=seg_all[:, b, :],
                                    in_=t3[:].rearrange("p (j k) -> p j k", k=K),
                                    axis=mybir.AxisListType.X,
                                    op=mybir.AluOpType.max)

        def stage_d(b):
            g, h = b // 2, b % 2
            tcols = slice(h * 128, (h + 1) * 128)
            obj_ps = ps_ob.tile([128, O_DIM], f32, tag="ob")
            for ki in range(KI):
                nc.tensor.matmul(obj_ps[:], x16_sb[:, g, ki, tcols],
                                 w_o16_sb[:, ki, :],
                                 start=(ki == 0), stop=(ki == KI - 1))
            obj_sb = work.tile([128, O_DIM], f32, tag="obj_sb")
            nc.vector.tensor_add(obj_sb[:], obj_ps[:], bias_bc[:, :O_DIM])
            nc.sync.dma_start(obj_d[b], obj_sb[:])

        for b in range(BPC + 2):
            if b < BPC:
                stage_a(b)
            if b % 2 == 1 and b // 2 < 4:
                g = b // 2
                nc.scalar.copy(x16_sb[:, g], inpT_sb[:, g])
            if 1 <= b < BPC + 1:
                stage_b(b - 1)
            if b >= 2:
                stage_c(b - 2)
            if 2 <= b < BPC + 2:
                stage_d(b - 2)
            if b == BPC:
                # non-exist head, class-sharded: ne[all 64 b, 256 shard].
                # Emitted late so its PE chain fills the pipeline drain (its
                # weights also arrive last in the DMA queue).
                ne_ps = ps_ne.tile([B, CSH], f32)
                nc.tensor.matmul(ne_ps[:], ones16[:], b_ne_sb[:],
                                 start=True, stop=False)
                for ki in range(KI):
                    nc.tensor.matmul(ne_ps[:], pooledT_sb[:, ki, :],
                                     w_ne_sb[:, ki, :],
                                     start=False, stop=(ki == KI - 1))
                ne_sb = work.tile([B, CSH], f32)
                nc.scalar.copy(ne_sb[:], ne_ps[:])
                nc.sync.dma_start(ne_d[:], ne_sb[:])
        nc.sync.dma_start(seg_d[:], seg_all[:])

    nc.compile()
    return nc


def _prep_host(inputs):
    inp = np.ascontiguousarray(inputs["inp"], dtype=np.float32)
    objmask = np.asarray(inputs["objmask"], dtype=np.float32)
    AA = np.asarray(inputs["AAidxs_tgts"]).astype(np.int64)
    W_obj = np.asarray(inputs["W_obj"], dtype=np.float32)
    b_obj = np.asarray(inputs["b_obj"], dtype=np.float32)
    W_act = np.asarray(inputs["W_act"], dtype=np.float32)
    b_act = np.asarray(inputs["b_act"], dtype=np.float32)
    W_ne = np.asarray(inputs["W_ne"], dtype=np.float32)
    b_ne = np.asarray(inputs["b_ne"], dtype=np.float32)

    ph1_idx, ph3_idx, S, K, pos = _build_routing(objmask, AA)

    # weights, chunked [KI, 128, cols]
    w_aT = np.ascontiguousarray(
        W_act.T.reshape(KI, 128, A_DIM).transpose(1, 0, 2))
    w_o16 = np.ascontiguousarray(
        W_obj.T.astype(np.float16).reshape(KI, 128, O_DIM).transpose(1, 0, 2))
    b_oa = np.concatenate([b_obj, b_act])[None, :]

    # normalized-mask pooling on host (pure input function)
    nmask = objmask / objmask.sum(axis=1, keepdims=True)
    pooled = np.einsum("bmd,bm->bd", inp, nmask).astype(np.float32)
    pooledT = np.ascontiguousarray(
        pooled.T.reshape(KI, 128, B).transpose(1, 0, 2)).astype(np.float16)

    in_maps = []
    for c in range(NCORES):
        bs = slice(c * BPC, (c + 1) * BPC)
        cs = slice(c * (C_DIM // NCORES), (c + 1) * (C_DIM // NCORES))
        inpT = inp[bs].reshape(BPC * 128, D).T.reshape(KI, 128, 4, 256)
        inpT = np.ascontiguousarray(inpT.transpose(1, 2, 0, 3))
        w_neT = np.zeros((128, KI, CSH), np.float16)
        w_neT[:, :, :C_DIM // NCORES] = W_ne[cs].T.astype(np.float16).reshape(
            KI, 128, C_DIM // NCORES).transpose(1, 0, 2)
        b_ne_p = np.zeros((1, CSH), np.float16)
        b_ne_p[0, :C_DIM // NCORES] = b_ne[cs].astype(np.float16)
        in_maps.append({
            "inpT": inpT,
            "w_aT": w_aT,
            "w_o16": w_o16,
            "b_oa": b_oa,
            "pooledT": pooledT,
            "w_neT": w_neT,
            "b_ne": b_ne_p,
            "ph1": np.ascontiguousarray(ph1_idx[bs].transpose(1, 0, 2)),
            "ph3": np.ascontiguousarray(ph3_idx[bs].transpose(1, 0, 2)),
        })
    return in_maps, S, K, pos


def _assemble(outs, pos):
    """Combine per-core results into full outputs (+ host where-combine)."""
    obj_out = np.concatenate([np.asarray(o["obj_out"]) for o in outs])
    act_out = np.concatenate([np.asarray(o["act_out"]) for o in outs])
    # seg_out[p, b, j] holds the class at permuted position j*128 + p
    seg_pos = np.concatenate(
        [np.transpose(np.asarray(o["seg_out"]), (1, 2, 0)).reshape(BPC, 2048)
         for o in outs])
    seg = seg_pos[:, pos]
    ne = np.concatenate(
        [np.asarray(o["ne_out"])[:, :C_DIM // NCORES] for o in outs], axis=1)
    final = np.where(seg > 0, seg, ne).astype(np.float32)
    return final, act_out, obj_out


def kernel(**inputs) -> tuple:
    from concourse.bass_utils import run_bass_kernel_spmd

    in_maps, S, K, pos = _prep_host(inputs)
    key = (S, K)
    if key not in _prog_cache:
        _prog_cache[key] = _build_program(S, K)
    nc = _prog_cache[key]

    res = run_bass_kernel_spmd(nc, in_maps, core_ids=list(range(NCORES)))
    return _assemble(res.results, pos)
